# revision 13
# baseline (speedup 1.0000x reference)
"""Trainium2 Bass kernel for a dense-transformer attention block.

Reference semantics (T=2048, D=2048, 16 heads, d_h=128):
    h = RMSNorm(x) * ln_w
    q,k,v = h @ W{q,k,v}.T  -> (n_h, T, d_h);  RoPE(q, k)
    att = softmax(causal(q k^T / sqrt(d_h))) @ v
    out = x + att @ Wo.T          (attention_mask is all-ones per spec)

Distribution: head-parallel over 8 cores (2 heads/core).  Each core:
  phase 1  QKV projections for its heads (bf16 matmuls, contract over d_model);
           RMSNorm folded in: row scales r[t] enter via r-scaled RoPE tables
           (q,k) and per-row scaling (v); ln_w is folded into the weights.
           rotate_half runs on the PE as a constant permutation matmul.
           x^2 row-sums use fp8 squares + DoubleRow ones-matmuls (2 k-tiles
           per pass).
  phase 2  per-head causal attention with scores computed TRANSPOSED
           (S^T[j,i]) so no transposes are needed anywhere; probabilities
           exp to fp8 (exp biased by -ln 8 to stay inside e4m3 range; the
           8x cancels in the softmax normalization); softmax row-sums
           accumulate on the PE via fp8 DoubleRow ones-matmuls; A@V keeps
           bf16 V against the fp8 probabilities.
  phase 3  per-head AllGather of att^T rows (overlaps the other head's work)
  phase 4  output projection column-shard, weight-stationary:
           out^T[:, cols_c] rows = sum_k WoT-chunk.T @ attT-chunk  + residual
           Interleaved into the main loop one block behind the collective so
           the AllGather latency hides under later blocks' compute.
Host assembles out = concat(out_colsT.T, axis=1).
"""

import math

import numpy as np

EPS = 1e-5
NEG = -1.0e30

CFG_FULL = dict(T=2048, D=2048, n_cores=8, heads_per_core=2)


# --------------------------------------------------------------------------
# device program
# --------------------------------------------------------------------------
def build_nc(T, D, n_cores, heads_per_core):
    import concourse.mybir as mybir
    import concourse.tile as tile
    from concourse import bacc

    DH = 128                      # head dim (hard-wired into layout)
    P = 128                       # partitions
    NH = heads_per_core
    DL = NH * DH                  # local width (q/k/v columns per core)
    KC = D // P                   # k-chunks over d_model
    TB = T // 512                 # 512-wide t blocks
    NIB = T // 512                # 512-wide i blocks
    NTS = T // P                  # 128-wide t subtiles
    f32 = mybir.dt.float32
    bf16 = mybir.dt.bfloat16
    fp8 = mybir.dt.float8e4
    i32 = mybir.dt.int32

    nc = bacc.Bacc("TRN2", target_bir_lowering=False, debug=False,
                   num_devices=n_cores)

    # ---- I/O ----
    xT = nc.dram_tensor("xT", [D, T], bf16, kind="ExternalInput").ap()
    xct_in = nc.dram_tensor("x_colsT", [DL, T], f32, kind="ExternalInput").ap()
    # weight tensors arrive host-pretiled in SBUF layout [P, KC*DL]
    wq_t = nc.dram_tensor("wq_t", [P, KC * DL], bf16, kind="ExternalInput").ap()
    wk_t = nc.dram_tensor("wk_t", [P, KC * DL], bf16, kind="ExternalInput").ap()
    wv_t = nc.dram_tensor("wv_t", [P, KC * DL], bf16, kind="ExternalInput").ap()
    # wo_t additionally row-permuted on host to the AllGather chunk order
    wo_t = nc.dram_tensor("wo_t", [P, KC * DL], bf16, kind="ExternalInput").ap()
    cosT = nc.dram_tensor("cosT", [DH, T], bf16, kind="ExternalInput").ap()
    sinT = nc.dram_tensor("sinT", [DH, T], bf16, kind="ExternalInput").ap()
    rot_t = nc.dram_tensor("rot_t", [DH, DH], bf16, kind="ExternalInput").ap()
    out_cT = nc.dram_tensor("out_colsT", [DL, T], f32,
                            kind="ExternalOutput").ap()

    Act = mybir.ActivationFunctionType
    Alu = mybir.AluOpType
    DR = mybir.MatmulPerfMode.DoubleRow
    inv_sqrt_dh = 1.0 / math.sqrt(DH)
    EXP_BIAS = -math.log(32.0)    # keeps exp() inside fp8 e4m3 range
    MAGIC = 0x5F3759DF

    with tile.TileContext(nc) as tc, \
            tc.tile_pool(name="persist", bufs=1) as persist:
        # ---------------- long-lived tensors ----------------
        Q_sb = persist.tile([P, NH, T], bf16, tag="Q_sb")
        K_sb = persist.tile([P, NH, T], bf16, tag="K_sb")
        V_sb = persist.tile([P, NTS, DL], bf16, tag="V_sb")
        rcol_sb = persist.tile([P, NTS], f32, tag="rcol_sb")
        rrow_sb = persist.tile([1, T], f32, tag="rrow_sb")
        ones8 = persist.tile([P, 2, 16], fp8, tag="ones8")
        masks_sb = persist.tile([P, 4, 512], f32, tag="masks_sb")
        rot_sb = persist.tile([P, DH], bf16, tag="rot_sb")
        ebias_sb = persist.tile([P, 1], f32, tag="ebias_sb")

        nc.gpsimd.dma_start(rot_sb[:], rot_t)
        nc.vector.memset(ebias_sb[:], EXP_BIAS)
        nc.vector.memset(ones8[:], 1.0)
        warm_sb = persist.tile([P, 128], bf16, tag="warm_sb")
        nc.vector.memset(warm_sb[:], 0.0)
        nc.gpsimd.memset(masks_sb[:], 0.0)
        for r in range(4):
            # keep (0) where i - j >= 0 with i = 512*B + f, j = 128*J + p,
            # offset r = J - 4*B  ->  f - p - 128 r >= 0
            nc.gpsimd.affine_select(
                out=masks_sb[:, r, :], in_=masks_sb[:, r, :],
                pattern=[[1, 512]], channel_multiplier=-1, base=-128 * r,
                compare_op=Alu.is_ge, fill=NEG)

        with tc.tile_pool(name="dram", bufs=1, space="DRAM") as dram_pool:
            ag_shared = "Shared" if n_cores > 4 else "Local"
            ag_in = [[dram_pool.tile([DH, 512], bf16, tag=f"agi{h}_{b}",
                                     name=f"ag_in{h}_{b}")
                      for b in range(NIB)] for h in range(NH)]
            ag_out = [[dram_pool.tile([n_cores * DH, 512], bf16,
                                      addr_space=ag_shared, tag=f"ago{h}_{b}",
                                      name=f"ag_out{h}_{b}")
                       for b in range(NIB)] for h in range(NH)]

            # PE warmup: ~5us of back-to-back dummy matmuls so the HAM
            # clock gate opens before the real work arrives
            with tc.tile_pool(name="warm_ps", bufs=1, space="PSUM") as wmps:
                wps = wmps.tile([P, 128], f32, tag="wm")
                for _ in range(40):
                    nc.tensor.matmul(wps[:], warm_sb[:], warm_sb[:],
                                     start=True, stop=True)

            # ==== phases 1+2+4 interleaved per t-block: QKV projections,
            # attention, the per-(head,block) all-gather, and the output
            # projection for the previous block — so the collective stream
            # starts early and its latency hides under compute.
            with (
                tc.tile_pool(name="wqkv", bufs=1) as wpool,
                tc.tile_pool(name="cs_raw", bufs=1) as cspool,
                tc.tile_pool(name="xk", bufs=1) as xpool,
                tc.tile_pool(name="sq", bufs=2) as sqpool,
                tc.tile_pool(name="tmp1", bufs=2) as tmppool,
                tc.tile_pool(name="rbc1", bufs=2) as rbcpool,
                tc.tile_pool(name="pt", bufs=3) as ptpool,
                tc.tile_pool(name="fin", bufs=2) as finpool,
                tc.tile_pool(name="ag_sb", bufs=8) as agpool,
                tc.tile_pool(name="xc", bufs=2) as xcpool,
                tc.tile_pool(name="osb", bufs=2) as opool,
                # PSUM: 4 + 3 + 1 banks (of 8)
                tc.tile_pool(name="big_ps", bufs=2, space="PSUM") as bigps,
                tc.tile_pool(name="sm_ps", bufs=3, space="PSUM") as smps,
                tc.tile_pool(name="row_ps", bufs=1, space="PSUM") as rowps,
            ):
                wq_sb = wpool.tile([P, KC, DL], bf16, tag="wq")
                wk_sb = wpool.tile([P, KC, DL], bf16, tag="wk")
                wv_sb = wpool.tile([P, KC, DL], bf16, tag="wv")
                wo_sb = wpool.tile([P, KC, DL], bf16, tag="wo")
                # interleave weight / x^T loads so the first q/k matmul can
                # start as soon as wq + xk[0] have landed
                xk = [xpool.tile([P, T], bf16, tag=f"xk{kc}", name=f"xk{kc}")
                      for kc in range(KC)]
                nc.sync.dma_start(wq_sb[:], wq_t.rearrange("p (kc j) -> p kc j", j=DL))
                for kc in range(KC):
                    eng = nc.sync if kc % 2 == 0 else nc.gpsimd
                    eng.dma_start(xk[kc][:], xT[P * kc:P * (kc + 1), :])
                nc.sync.dma_start(wk_sb[:], wk_t.rearrange("p (kc j) -> p kc j", j=DL))
                nc.sync.dma_start(wv_sb[:], wv_t.rearrange("p (kc j) -> p kc j", j=DL))
                nc.gpsimd.dma_start(wo_sb[:], wo_t.rearrange("p (kc j) -> p kc j", j=DL))
                # cos/sin tables; r is folded in per block, in place
                cos_r = cspool.tile([P, T], bf16, tag="cos")
                sin_r = cspool.tile([P, T], bf16, tag="sin")
                nc.sync.dma_start(cos_r[:], cosT)
                nc.sync.dma_start(sin_r[:], sinT)

                # ---------- phase 4 for block B (output projection) -------
                def phase4(B, halves=(0, 1)):
                    sl = slice(512 * B, 512 * B + 512)
                    oms = {}
                    for half in halves:
                        ags = []
                        for kc in range(8 * half, 8 * half + 8):
                            h_idx, c_idx = divmod(kc, KC // NH)
                            agt = agpool.tile([P, 512], bf16, tag="ag",
                                              name=f"ag{kc}_{B}")
                            eng = (nc.sync, nc.gpsimd,
                                   nc.scalar)[kc % 3]
                            eng.dma_start(
                                agt[:],
                                ag_out[h_idx][B][P * c_idx:P * (c_idx + 1), :])
                            ags.append(agt)
                        for js in range(DL // P):
                            if half == halves[0]:
                                oms[js] = smps.tile([P, 512], f32, tag="sm",
                                                    name=f"om{js}_{B}")
                            om = oms[js]
                            for i, kc in enumerate(range(8 * half,
                                                         8 * half + 8)):
                                nc.tensor.matmul(
                                    om[:], wo_sb[:, kc, P * js:P * (js + 1)],
                                    ags[i][:],
                                    start=(half == halves[0] and i == 0),
                                    stop=(half == halves[-1] and i == 7))
                        if half != halves[-1]:
                            continue
                        for js in range(DL // P):
                            om = oms[js]
                            xct = xcpool.tile([P, 512], f32, tag="xct")
                            nc.scalar.dma_start(xct[:],
                                                xct_in[P * js:P * (js + 1), sl])
                            osb = opool.tile([P, 512], f32, tag="osb")
                            nc.vector.tensor_tensor(osb[:], om[:], xct[:],
                                                    Alu.add)
                            nc.sync.dma_start(out_cT[P * js:P * (js + 1), sl],
                                                osb[:])

                # ---------- phase 2 for block B, one head ----------
                def phase2_head(B, h):
                    ib = slice(512 * B, 512 * B + 512)
                    hs = slice(DH * h, DH * (h + 1))
                    av = smps.tile([P, 512], f32, tag="sm")
                    ssum = rowps.tile([1, 512], f32, tag="row")
                    Jmax = 4 * B + 3
                    for Jp in range(0, Jmax + 1, 2):
                        st = bigps.tile([P, 2, 512], f32, tag="big")
                        pt = ptpool.tile([P, 2, 512], fp8, tag="pt")
                        for gi in range(2):
                            J = Jp + gi
                            nc.tensor.matmul(st[:, gi, :],
                                             K_sb[:, h, P * J:P * (J + 1)],
                                             Q_sb[:, h, ib],
                                             start=True, stop=True)
                            if J // 4 == B:
                                nc.vector.tensor_tensor(
                                    st[:, gi, :], st[:, gi, :],
                                    masks_sb[:, J % 4, :], Alu.add)
                        nc.scalar.activation(pt[:], st[:], Act.Exp,
                                             scale=inv_sqrt_dh,
                                             bias=ebias_sb[:])
                        for gi in range(2):
                            J = Jp + gi
                            nc.tensor.matmul(av[:], V_sb[:, J, hs],
                                             pt[:, gi, :], start=(J == 0),
                                             stop=(J == Jmax))
                        nc.tensor.matmul(ssum[:], ones8[:, :, 0:1], pt[:, 0:2, :],
                                         start=(Jp == 0), stop=(Jp == Jmax - 1),
                                         perf_mode=DR)
                    rinv = finpool.tile([1, 512], f32, tag="rinv")
                    nc.vector.reciprocal_approx_fast(rinv[:], ssum[:])
                    rb = finpool.tile([P, 512], f32, tag="rb")
                    nc.gpsimd.partition_broadcast(rb[:], rinv[:])
                    att = finpool.tile([P, 512], bf16, tag="att")
                    nc.vector.tensor_tensor(att[:], av[:], rb[:], Alu.mult)
                    nc.sync.dma_start(ag_in[h][B][:], att[:])
                    nc.gpsimd.collective_compute(
                        "AllGather", Alu.bypass,
                        replica_groups=[list(range(n_cores))],
                        ins=[ag_in[h][B][:].opt()],
                        outs=[ag_out[h][B][:].opt()])

                for B in range(TB):
                    tb = slice(512 * B, 512 * B + 512)
                    # ---------- phase 1 for block B ----------
                    srow = rowps.tile([1, 512], f32, tag="row")
                    qps = bigps.tile([P, 2, 512], f32, tag="big")
                    for kc2 in range(0, KC, 2):
                        sq = sqpool.tile([P, 2, 512], fp8, tag="sq")
                        for gi in range(2):
                            nc.scalar.activation(sq[:, gi, :],
                                                 xk[kc2 + gi][:, tb],
                                                 Act.Square)
                        nc.tensor.matmul(srow[:], ones8[:, :, 0:1], sq[:],
                                         start=(kc2 == 0),
                                         stop=(kc2 == KC - 2), perf_mode=DR)
                        for kc in (kc2, kc2 + 1):
                            for h in range(NH):
                                hs = slice(DH * h, DH * (h + 1))
                                nc.tensor.matmul(qps[:, h, :],
                                                 wq_sb[:, kc, hs],
                                                 xk[kc][:, tb],
                                                 start=(kc == 0),
                                                 stop=(kc == KC - 1))
                    for h in range(NH):
                        nc.vector.tensor_copy(Q_sb[:, h, tb], qps[:, h, :])
                    # r = rsqrt(mean + eps): bit-trick seed + 2 Newton (DVE)
                    rr = rrow_sb[0:1, tb]
                    mrow = tmppool.tile([1, 512], f32, tag="mrow")
                    nc.vector.tensor_scalar(mrow[:], srow[:], 1.0 / D, EPS,
                                            Alu.mult, Alu.add)
                    ri = tmppool.tile([1, 512], i32, tag="ri")
                    nc.vector.tensor_scalar(ri[:], mrow[:].bitcast(i32), 1, None,
                                            Alu.arith_shift_right)
                    nc.vector.tensor_scalar(ri[:], ri[:], -1, MAGIC,
                                            Alu.mult, Alu.add)
                    rrv = ri[:].bitcast(f32)
                    tn = tmppool.tile([1, 512], f32, tag="tn")
                    nc.vector.tensor_tensor(tn[:], rrv, rrv, Alu.mult)
                    nc.vector.tensor_tensor(tn[:], tn[:], mrow[:], Alu.mult)
                    nc.vector.tensor_scalar(tn[:], tn[:], -0.5, 1.5,
                                            Alu.mult, Alu.add)
                    nc.vector.tensor_tensor(rrv, rrv, tn[:], Alu.mult)
                    nc.vector.tensor_tensor(tn[:], rrv, rrv, Alu.mult)
                    nc.vector.tensor_tensor(tn[:], tn[:], mrow[:], Alu.mult)
                    nc.vector.tensor_scalar(tn[:], tn[:], -0.5, 1.5,
                                            Alu.mult, Alu.add)
                    nc.vector.tensor_tensor(rr, rrv, tn[:], Alu.mult)
                    rbc = rbcpool.tile([P, 512], f32, tag="rbc")
                    nc.gpsimd.partition_broadcast(rbc[:], rr)
                    for s in range(4):
                        i = 4 * B + s
                        nc.gpsimd.dma_start(
                            out=rcol_sb[:, i:i + 1],
                            in_=rrow_sb[0:1, 512 * B + 128 * s:
                                        512 * B + 128 * (s + 1)])
                    nc.vector.tensor_tensor(cos_r[:, tb], cos_r[:, tb], rbc[:], Alu.mult)
                    nc.vector.tensor_tensor(sin_r[:, tb], sin_r[:, tb], rbc[:], Alu.mult)
                    # K pass (second big-psum buffer; overlaps the Q drain)
                    kps = bigps.tile([P, 2, 512], f32, tag="big")
                    for kc in range(KC):
                        for h in range(NH):
                            hs = slice(DH * h, DH * (h + 1))
                            nc.tensor.matmul(kps[:, h, :], wk_sb[:, kc, hs],
                                             xk[kc][:, tb], start=(kc == 0),
                                             stop=(kc == KC - 1))
                    for h in range(NH):
                        nc.vector.tensor_copy(K_sb[:, h, tb], kps[:, h, :])
                    # V pass, one 512-row tile (1 psum bank) at a time
                    for ts in range(4):
                        i = 4 * B + ts
                        vp = smps.tile([P, 512], f32, tag="sm")
                        for kc in range(KC):
                            nc.tensor.matmul(vp[:, :DL],
                                             xk[kc][:, 512 * B + P * ts:
                                                    512 * B + P * (ts + 1)],
                                             wv_sb[:, kc, :], start=(kc == 0),
                                             stop=(kc == KC - 1))
                        nc.vector.tensor_copy(V_sb[:, i, :], vp[:, :DL])
                        nc.vector.tensor_scalar_mul(V_sb[:, i, :], V_sb[:, i, :],
                                                    rcol_sb[:, i:i + 1])
                    # RoPE in place on SBUF (r enters via the scaled tables)
                    for buf in (Q_sb, K_sb):
                        for h in range(NH):
                            qs = tmppool.tile([P, 512], bf16, tag="qs")
                            nc.vector.tensor_tensor(qs[:], buf[:, h, tb],
                                                    sin_r[:, tb], Alu.mult)
                            rps = smps.tile([P, 512], f32, tag="sm")
                            nc.tensor.matmul(rps[:], rot_sb[:], qs[:],
                                             start=True, stop=True)
                            nc.vector.tensor_tensor(buf[:, h, tb], buf[:, h, tb],
                                                    cos_r[:, tb], Alu.mult)
                            nc.vector.tensor_tensor(buf[:, h, tb], buf[:, h, tb],
                                                    rps[:], Alu.add)
                    # ---------- phase 2 + interleaved phase 4 ----------
                    phase2_head(B, 0)
                    if B == 2:
                        phase4(0)
                    phase2_head(B, 1)
                    if B == 2:
                        phase4(1)
                    elif B == 3:
                        phase4(2)
                # tail: only the last block's projection remains; its head-0
                # half queues ahead of the head-1 ag loads so it runs while
                # the second head's AllGather is still in flight
                phase4(TB - 1)

    nc.compile()
    return nc


# --------------------------------------------------------------------------
# host-side prep / entry point
# --------------------------------------------------------------------------
def prepare_inputs(x, cos, sin, ln_w, Wq, Wk, Wv, Wo, n_cores, heads_per_core):
    import ml_dtypes
    bf16 = ml_dtypes.bfloat16
    DH = 128
    DL = heads_per_core * DH
    x = np.ascontiguousarray(np.asarray(x, dtype=np.float32))
    cos = np.asarray(cos, dtype=np.float32)
    sin = np.asarray(sin, dtype=np.float32)
    ln_w = np.ascontiguousarray(np.asarray(ln_w, dtype=np.float32))
    xT = np.ascontiguousarray(x.T.astype(bf16))
    cosT = np.ascontiguousarray(cos.T.astype(bf16))
    sinT = np.ascontiguousarray(sin.T.astype(bf16))
    R = np.zeros((DH, DH), dtype=np.float32)
    R[np.arange(64), np.arange(64) + 64] = -1.0
    R[np.arange(64) + 64, np.arange(64)] = 1.0
    rot_t = np.ascontiguousarray(R.T.astype(bf16))
    # AllGather chunk order: head-major, then source core; each chunk is the
    # 128 att columns (global j = DL*c' + DH*h + d) that core c' / head h sent.
    perm = np.concatenate([
        DL * cp + DH * h + np.arange(DH)
        for h in range(heads_per_core) for cp in range(n_cores)
    ])
    D = x.shape[1]
    KC = D // DH

    def pretile(wT):
        # (D, DL) -> SBUF layout [P, KC*DL]: element (p, kc, j) = wT[128 kc + p, j]
        return np.ascontiguousarray(
            wT.reshape(KC, DH, DL).transpose(1, 0, 2).reshape(DH, KC * DL)
            .astype(bf16))

    in_maps = []
    for c in range(n_cores):
        cols = slice(c * DL, (c + 1) * DL)
        woT = np.asarray(Wo, np.float32)[cols, :].T  # (D, DL)
        in_maps.append({
            "xT": xT,
            "x_colsT": np.ascontiguousarray(x[:, cols].T),
            "wq_t": pretile((np.asarray(Wq, np.float32)[cols, :] * ln_w).T),
            "wk_t": pretile((np.asarray(Wk, np.float32)[cols, :] * ln_w).T),
            "wv_t": pretile((np.asarray(Wv, np.float32)[cols, :] * ln_w).T),
            "wo_t": pretile(woT[perm, :]),
            "cosT": cosT,
            "sinT": sinT,
            "rot_t": rot_t,
        })
    return in_maps


_NC_CACHE = {}


def kernel(x, cos, sin, attention_mask, ln_w, Wq, Wk, Wv, Wo,
           _trace=False, _trace_cores=None):
    from concourse.bass_utils import run_bass_kernel_spmd

    cfg = CFG_FULL
    key = tuple(sorted(cfg.items()))
    if key not in _NC_CACHE:
        _NC_CACHE[key] = build_nc(**cfg)
    nc = _NC_CACHE[key]
    n_cores = cfg["n_cores"]
    in_maps = prepare_inputs(x, cos, sin, ln_w, Wq, Wk, Wv, Wo,
                             n_cores, cfg["heads_per_core"])
    res = run_bass_kernel_spmd(nc, in_maps, core_ids=list(range(n_cores)),
                               trace=_trace, trace_cores=_trace_cores)
    out = np.concatenate(
        [res.results[c]["out_colsT"].T for c in range(n_cores)], axis=1)
    kernel.last_result = res
    return out


# revision 21
# speedup vs baseline: 1.0055x; 1.0055x over previous
"""Trainium2 Bass kernel for a dense-transformer attention block.

Reference semantics (T=2048, D=2048, 16 heads, d_h=128):
    h = RMSNorm(x) * ln_w
    q,k,v = h @ W{q,k,v}.T  -> (n_h, T, d_h);  RoPE(q, k)
    att = softmax(causal(q k^T / sqrt(d_h))) @ v
    out = x + att @ Wo.T          (attention_mask is all-ones per spec)

Distribution: head-parallel over 8 cores (2 heads/core).  Each core:
  phase 1  QKV projections for its heads (bf16 matmuls, contract over d_model);
           RMSNorm folded in: row scales r[t] enter via r-scaled RoPE tables
           (q,k) and per-row scaling (v); ln_w is folded into the weights.
           rotate_half runs on the PE as a constant permutation matmul.
           x^2 row-sums use fp8 squares + DoubleRow ones-matmuls (2 k-tiles
           per pass).
  phase 2  per-head causal attention with scores computed TRANSPOSED
           (S^T[j,i]) so no transposes are needed anywhere; probabilities
           exp to fp8 (exp biased by -ln 8 to stay inside e4m3 range; the
           8x cancels in the softmax normalization); softmax row-sums
           accumulate on the PE via fp8 DoubleRow ones-matmuls; A@V keeps
           bf16 V against the fp8 probabilities.
  phase 3  per-head AllGather of att^T rows (overlaps the other head's work)
  phase 4  output projection column-shard, weight-stationary:
           out^T[:, cols_c] rows = sum_k WoT-chunk.T @ attT-chunk  + residual
           Interleaved into the main loop one block behind the collective so
           the AllGather latency hides under later blocks' compute.
Host assembles out = concat(out_colsT.T, axis=1).
"""

import math

import numpy as np

EPS = 1e-5
NEG = -1.0e30

CFG_FULL = dict(T=2048, D=2048, n_cores=8, heads_per_core=2)


# --------------------------------------------------------------------------
# device program
# --------------------------------------------------------------------------
def build_nc(T, D, n_cores, heads_per_core):
    import concourse.mybir as mybir
    import concourse.tile as tile
    from concourse import bacc

    DH = 128                      # head dim (hard-wired into layout)
    P = 128                       # partitions
    NH = heads_per_core
    DL = NH * DH                  # local width (q/k/v columns per core)
    KC = D // P                   # k-chunks over d_model
    TB = T // 512                 # 512-wide t blocks
    NIB = T // 512                # 512-wide i blocks
    NTS = T // P                  # 128-wide t subtiles
    f32 = mybir.dt.float32
    bf16 = mybir.dt.bfloat16
    fp8 = mybir.dt.float8e4
    i32 = mybir.dt.int32

    nc = bacc.Bacc("TRN2", target_bir_lowering=False, debug=False,
                   num_devices=n_cores)

    # ---- I/O ----
    xT = nc.dram_tensor("xT", [D, T], bf16, kind="ExternalInput").ap()
    xct_in = nc.dram_tensor("x_colsT", [DL, T], f32, kind="ExternalInput").ap()
    # weight tensors arrive host-pretiled in SBUF layout [P, KC*DL]
    wq_t = nc.dram_tensor("wq_t", [P, KC * DL], bf16, kind="ExternalInput").ap()
    wk_t = nc.dram_tensor("wk_t", [P, KC * DL], bf16, kind="ExternalInput").ap()
    wv_t = nc.dram_tensor("wv_t", [P, KC * DL], bf16, kind="ExternalInput").ap()
    # wo_t additionally row-permuted on host to the AllGather chunk order
    wo_t = nc.dram_tensor("wo_t", [P, KC * DL], bf16, kind="ExternalInput").ap()
    cosT = nc.dram_tensor("cosT", [DH, T], bf16, kind="ExternalInput").ap()
    sinT = nc.dram_tensor("sinT", [DH, T], bf16, kind="ExternalInput").ap()
    rot_t = nc.dram_tensor("rot_t", [DH, DH], bf16, kind="ExternalInput").ap()
    out_cT = nc.dram_tensor("out_colsT", [DL, T], f32,
                            kind="ExternalOutput").ap()

    Act = mybir.ActivationFunctionType
    Alu = mybir.AluOpType
    DR = mybir.MatmulPerfMode.DoubleRow
    inv_sqrt_dh = 1.0 / math.sqrt(DH)
    EXP_BIAS = -math.log(32.0)    # keeps exp() inside fp8 e4m3 range
    MAGIC = 0x5F3759DF

    with tile.TileContext(nc) as tc, \
            tc.tile_pool(name="persist", bufs=1) as persist:
        # ---------------- long-lived tensors ----------------
        Q_sb = persist.tile([P, NH, T], bf16, tag="Q_sb")
        K_sb = persist.tile([P, NH, T], bf16, tag="K_sb")
        V_sb = persist.tile([P, NTS, DL], bf16, tag="V_sb")
        rcol_sb = persist.tile([P, NTS], f32, tag="rcol_sb")
        # wide ones: row-sum matmuls produce their result broadcast across
        # all 128 partitions for free (streaming is rhs-bound)
        ones8 = persist.tile([P, 2, P], fp8, tag="ones8")
        masks_sb = persist.tile([P, 4, 512], f32, tag="masks_sb")
        rot_sb = persist.tile([P, DH], bf16, tag="rot_sb")
        ebias_sb = persist.tile([P, 1], f32, tag="ebias_sb")

        nc.gpsimd.dma_start(rot_sb[:], rot_t)
        nc.vector.memset(ebias_sb[:], EXP_BIAS)
        nc.vector.memset(ones8[:], 1.0)
        warm_sb = persist.tile([P, 128], bf16, tag="warm_sb")
        nc.vector.memset(warm_sb[:], 0.0)
        nc.gpsimd.memset(masks_sb[:], 0.0)
        for r in range(4):
            # keep (0) where i - j >= 0 with i = 512*B + f, j = 128*J + p,
            # offset r = J - 4*B  ->  f - p - 128 r >= 0
            nc.gpsimd.affine_select(
                out=masks_sb[:, r, :], in_=masks_sb[:, r, :],
                pattern=[[1, 512]], channel_multiplier=-1, base=-128 * r,
                compare_op=Alu.is_ge, fill=NEG)

        with tc.tile_pool(name="dram", bufs=1, space="DRAM") as dram_pool:
            ag_shared = "Shared" if n_cores > 4 else "Local"
            ag_in = [[dram_pool.tile([DH, 512], bf16, tag=f"agi{h}_{b}",
                                     name=f"ag_in{h}_{b}")
                      for b in range(NIB)] for h in range(NH)]
            ag_out = [[dram_pool.tile([n_cores * DH, 512], bf16,
                                      addr_space=ag_shared, tag=f"ago{h}_{b}",
                                      name=f"ag_out{h}_{b}")
                       for b in range(NIB)] for h in range(NH)]

            # PE warmup: ~5us of back-to-back dummy matmuls so the HAM
            # clock gate opens before the real work arrives
            with tc.tile_pool(name="warm_ps", bufs=1, space="PSUM") as wmps:
                wps = wmps.tile([P, 128], f32, tag="wm")
                for _ in range(40):
                    nc.tensor.matmul(wps[:], warm_sb[:], warm_sb[:],
                                     start=True, stop=True)

            # ==== phases 1+2+4 interleaved per t-block: QKV projections,
            # attention, the per-(head,block) all-gather, and the output
            # projection for the previous block — so the collective stream
            # starts early and its latency hides under compute.
            with (
                tc.tile_pool(name="wqkv", bufs=1) as wpool,
                tc.tile_pool(name="cs_raw", bufs=1) as cspool,
                tc.tile_pool(name="xk", bufs=1) as xpool,
                tc.tile_pool(name="sq", bufs=2) as sqpool,
                tc.tile_pool(name="tmp1", bufs=2) as tmppool,
                tc.tile_pool(name="rbc1", bufs=2) as rbcpool,
                tc.tile_pool(name="pt", bufs=3) as ptpool,
                tc.tile_pool(name="fin", bufs=2) as finpool,
                tc.tile_pool(name="ag_sb", bufs=8) as agpool,
                tc.tile_pool(name="xc", bufs=2) as xcpool,
                tc.tile_pool(name="osb", bufs=2) as opool,
                # PSUM: 4 + 3 + 1 banks (of 8)
                tc.tile_pool(name="big_ps", bufs=2, space="PSUM") as bigps,
                tc.tile_pool(name="sm_ps", bufs=3, space="PSUM") as smps,
                tc.tile_pool(name="row_ps", bufs=1, space="PSUM") as rowps,
            ):
                wq_sb = wpool.tile([P, KC, DL], bf16, tag="wq")
                wk_sb = wpool.tile([P, KC, DL], bf16, tag="wk")
                wv_sb = wpool.tile([P, KC, DL], bf16, tag="wv")
                wo_sb = wpool.tile([P, KC, DL], bf16, tag="wo")
                # interleave weight / x^T loads so the first q/k matmul can
                # start as soon as wq + xk[0] have landed
                xk = [xpool.tile([P, T], bf16, tag=f"xk{kc}", name=f"xk{kc}")
                      for kc in range(KC)]
                nc.sync.dma_start(wq_sb[:], wq_t.rearrange("p (kc j) -> p kc j", j=DL))
                for kc in range(KC):
                    eng = nc.sync if kc % 2 == 0 else nc.gpsimd
                    eng.dma_start(xk[kc][:], xT[P * kc:P * (kc + 1), :])
                nc.sync.dma_start(wk_sb[:], wk_t.rearrange("p (kc j) -> p kc j", j=DL))
                nc.sync.dma_start(wv_sb[:], wv_t.rearrange("p (kc j) -> p kc j", j=DL))
                nc.gpsimd.dma_start(wo_sb[:], wo_t.rearrange("p (kc j) -> p kc j", j=DL))
                # cos/sin tables; r is folded in per block, in place
                cos_r = cspool.tile([P, T], bf16, tag="cos")
                sin_r = cspool.tile([P, T], bf16, tag="sin")
                nc.sync.dma_start(cos_r[:], cosT)
                nc.sync.dma_start(sin_r[:], sinT)

                # ---------- phase 4 for block B (output projection) -------
                def phase4(B, halves=(0, 1)):
                    sl = slice(512 * B, 512 * B + 512)
                    oms = {}
                    for half in halves:
                        ags = []
                        for kc in range(8 * half, 8 * half + 8):
                            h_idx, c_idx = divmod(kc, KC // NH)
                            agt = agpool.tile([P, 512], bf16, tag="ag",
                                              name=f"ag{kc}_{B}")
                            eng = (nc.sync, nc.scalar)[kc % 2]
                            eng.dma_start(
                                agt[:],
                                ag_out[h_idx][B][P * c_idx:P * (c_idx + 1), :])
                            ags.append(agt)
                        for js in range(DL // P):
                            if half == halves[0]:
                                oms[js] = smps.tile([P, 512], f32, tag="sm",
                                                    name=f"om{js}_{B}")
                            om = oms[js]
                            for i, kc in enumerate(range(8 * half,
                                                         8 * half + 8)):
                                nc.tensor.matmul(
                                    om[:], wo_sb[:, kc, P * js:P * (js + 1)],
                                    ags[i][:],
                                    start=(half == halves[0] and i == 0),
                                    stop=(half == halves[-1] and i == 7))
                        if half != halves[-1]:
                            continue
                        for js in range(DL // P):
                            om = oms[js]
                            xct = xcpool.tile([P, 512], f32, tag="xct")
                            nc.scalar.dma_start(xct[:],
                                                xct_in[P * js:P * (js + 1), sl])
                            osb = opool.tile([P, 512], f32, tag="osb")
                            nc.vector.tensor_tensor(osb[:], om[:], xct[:],
                                                    Alu.add)
                            nc.sync.dma_start(out_cT[P * js:P * (js + 1), sl],
                                                osb[:])

                # ---------- phase 2 for block B, one head ----------
                def phase2_head(B, h):
                    ib = slice(512 * B, 512 * B + 512)
                    hs = slice(DH * h, DH * (h + 1))
                    av = smps.tile([P, 512], f32, tag="sm")
                    ssum = rowps.tile([P, 512], f32, tag="row")
                    Jmax = 4 * B + 3
                    for Jp in range(0, Jmax + 1, 2):
                        st = bigps.tile([P, 2, 512], f32, tag="big")
                        pt = ptpool.tile([P, 2, 512], fp8, tag="pt")
                        for gi in range(2):
                            J = Jp + gi
                            nc.tensor.matmul(st[:, gi, :],
                                             K_sb[:, h, P * J:P * (J + 1)],
                                             Q_sb[:, h, ib],
                                             start=True, stop=True)
                            if J // 4 == B:
                                nc.vector.tensor_tensor(
                                    st[:, gi, :], st[:, gi, :],
                                    masks_sb[:, J % 4, :], Alu.add)
                        nc.scalar.activation(pt[:], st[:], Act.Exp,
                                             scale=inv_sqrt_dh,
                                             bias=ebias_sb[:])
                        for gi in range(2):
                            J = Jp + gi
                            nc.tensor.matmul(av[:], V_sb[:, J, hs],
                                             pt[:, gi, :], start=(J == 0),
                                             stop=(J == Jmax))
                        nc.tensor.matmul(ssum[:], ones8[:], pt[:, 0:2, :],
                                         start=(Jp == 0), stop=(Jp == Jmax - 1),
                                         perf_mode=DR)
                    rinv = finpool.tile([P, 512], f32, tag="rinv")
                    nc.vector.reciprocal_approx_fast(rinv[:], ssum[:])
                    att = finpool.tile([P, 512], bf16, tag="att")
                    nc.vector.tensor_tensor(att[:], av[:], rinv[:], Alu.mult)
                    nc.sync.dma_start(ag_in[h][B][:], att[:])
                    nc.gpsimd.collective_compute(
                        "AllGather", Alu.bypass,
                        replica_groups=[list(range(n_cores))],
                        ins=[ag_in[h][B][:].opt()],
                        outs=[ag_out[h][B][:].opt()])

                for B in range(TB):
                    tb = slice(512 * B, 512 * B + 512)
                    # ---------- phase 1 for block B ----------
                    srow = rowps.tile([P, 512], f32, tag="row")
                    qps = bigps.tile([P, 2, 512], f32, tag="big")
                    for kc2 in range(0, KC, 2):
                        sq = sqpool.tile([P, 2, 512], fp8, tag="sq")
                        for gi in range(2):
                            nc.scalar.activation(sq[:, gi, :],
                                                 xk[kc2 + gi][:, tb],
                                                 Act.Square)
                        nc.tensor.matmul(srow[:], ones8[:], sq[:],
                                         start=(kc2 == 0),
                                         stop=(kc2 == KC - 2), perf_mode=DR)
                        for kc in (kc2, kc2 + 1):
                            for h in range(NH):
                                hs = slice(DH * h, DH * (h + 1))
                                nc.tensor.matmul(qps[:, h, :],
                                                 wq_sb[:, kc, hs],
                                                 xk[kc][:, tb],
                                                 start=(kc == 0),
                                                 stop=(kc == KC - 1))
                    for h in range(NH):
                        nc.vector.tensor_copy(Q_sb[:, h, tb], qps[:, h, :])
                    # r = rsqrt(mean + eps): bit-trick seed + 2 Newton (DVE);
                    # computed on all 128 partitions (srow arrives broadcast)
                    # so the result needs no partition broadcast anywhere
                    mrow = tmppool.tile([P, 512], f32, tag="mrow")
                    nc.vector.tensor_scalar(mrow[:], srow[:], 1.0 / D, EPS,
                                            Alu.mult, Alu.add)
                    ri = tmppool.tile([P, 512], i32, tag="ri")
                    nc.vector.tensor_scalar(ri[:], mrow[:].bitcast(i32), 1, None,
                                            Alu.arith_shift_right)
                    nc.vector.tensor_scalar(ri[:], ri[:], -1, MAGIC,
                                            Alu.mult, Alu.add)
                    rrv = ri[:].bitcast(f32)
                    tn = tmppool.tile([P, 512], f32, tag="tn")
                    nc.vector.tensor_tensor(tn[:], rrv, rrv, Alu.mult)
                    nc.vector.tensor_tensor(tn[:], tn[:], mrow[:], Alu.mult)
                    nc.vector.tensor_scalar(tn[:], tn[:], -0.5, 1.5,
                                            Alu.mult, Alu.add)
                    nc.vector.tensor_tensor(rrv, rrv, tn[:], Alu.mult)
                    nc.vector.tensor_tensor(tn[:], rrv, rrv, Alu.mult)
                    nc.vector.tensor_tensor(tn[:], tn[:], mrow[:], Alu.mult)
                    nc.vector.tensor_scalar(tn[:], tn[:], -0.5, 1.5,
                                            Alu.mult, Alu.add)
                    rbc = rbcpool.tile([P, 512], f32, tag="rbc")
                    nc.vector.tensor_tensor(rbc[:], rrv, tn[:], Alu.mult)
                    for s in range(4):
                        i = 4 * B + s
                        nc.scalar.dma_start(
                            out=rcol_sb[:, i:i + 1],
                            in_=rbc[0:1, 128 * s:128 * (s + 1)])
                    nc.vector.tensor_tensor(cos_r[:, tb], cos_r[:, tb], rbc[:], Alu.mult)
                    nc.vector.tensor_tensor(sin_r[:, tb], sin_r[:, tb], rbc[:], Alu.mult)
                    # K pass (second big-psum buffer; overlaps the Q drain)
                    kps = bigps.tile([P, 2, 512], f32, tag="big")
                    for kc in range(KC):
                        for h in range(NH):
                            hs = slice(DH * h, DH * (h + 1))
                            nc.tensor.matmul(kps[:, h, :], wk_sb[:, kc, hs],
                                             xk[kc][:, tb], start=(kc == 0),
                                             stop=(kc == KC - 1))
                    for h in range(NH):
                        nc.vector.tensor_copy(K_sb[:, h, tb], kps[:, h, :])
                    # V pass, one 512-row tile (1 psum bank) at a time
                    for ts in range(4):
                        i = 4 * B + ts
                        vp = smps.tile([P, 512], f32, tag="sm")
                        for kc in range(KC):
                            nc.tensor.matmul(vp[:, :DL],
                                             xk[kc][:, 512 * B + P * ts:
                                                    512 * B + P * (ts + 1)],
                                             wv_sb[:, kc, :], start=(kc == 0),
                                             stop=(kc == KC - 1))
                        nc.vector.tensor_copy(V_sb[:, i, :], vp[:, :DL])
                        nc.vector.tensor_scalar_mul(V_sb[:, i, :], V_sb[:, i, :],
                                                    rcol_sb[:, i:i + 1])
                    # RoPE in place on SBUF (r enters via the scaled tables)
                    for buf in (Q_sb, K_sb):
                        for h in range(NH):
                            qs = tmppool.tile([P, 512], bf16, tag="qs")
                            nc.vector.tensor_tensor(qs[:], buf[:, h, tb],
                                                    sin_r[:, tb], Alu.mult)
                            rps = smps.tile([P, 512], f32, tag="sm")
                            nc.tensor.matmul(rps[:], rot_sb[:], qs[:],
                                             start=True, stop=True)
                            nc.vector.tensor_tensor(buf[:, h, tb], buf[:, h, tb],
                                                    cos_r[:, tb], Alu.mult)
                            nc.vector.tensor_tensor(buf[:, h, tb], buf[:, h, tb],
                                                    rps[:], Alu.add)
                    # ---------- phase 2 + interleaved phase 4 ----------
                    phase2_head(B, 0)
                    if B == 2:
                        phase4(0)
                    phase2_head(B, 1)
                    if B == 2:
                        phase4(1)
                    elif B == 3:
                        phase4(2)
                # tail: only the last block's projection remains; its head-0
                # half queues ahead of the head-1 ag loads so it runs while
                # the second head's AllGather is still in flight
                phase4(TB - 1)

    nc.compile()
    return nc


# --------------------------------------------------------------------------
# host-side prep / entry point
# --------------------------------------------------------------------------
def prepare_inputs(x, cos, sin, ln_w, Wq, Wk, Wv, Wo, n_cores, heads_per_core):
    import ml_dtypes
    bf16 = ml_dtypes.bfloat16
    DH = 128
    DL = heads_per_core * DH
    x = np.ascontiguousarray(np.asarray(x, dtype=np.float32))
    cos = np.asarray(cos, dtype=np.float32)
    sin = np.asarray(sin, dtype=np.float32)
    ln_w = np.ascontiguousarray(np.asarray(ln_w, dtype=np.float32))
    xT = np.ascontiguousarray(x.T.astype(bf16))
    cosT = np.ascontiguousarray(cos.T.astype(bf16))
    sinT = np.ascontiguousarray(sin.T.astype(bf16))
    R = np.zeros((DH, DH), dtype=np.float32)
    R[np.arange(64), np.arange(64) + 64] = -1.0
    R[np.arange(64) + 64, np.arange(64)] = 1.0
    rot_t = np.ascontiguousarray(R.T.astype(bf16))
    # AllGather chunk order: head-major, then source core; each chunk is the
    # 128 att columns (global j = DL*c' + DH*h + d) that core c' / head h sent.
    perm = np.concatenate([
        DL * cp + DH * h + np.arange(DH)
        for h in range(heads_per_core) for cp in range(n_cores)
    ])
    D = x.shape[1]
    KC = D // DH

    def pretile(wT):
        # (D, DL) -> SBUF layout [P, KC*DL]: element (p, kc, j) = wT[128 kc + p, j]
        return np.ascontiguousarray(
            wT.reshape(KC, DH, DL).transpose(1, 0, 2).reshape(DH, KC * DL)
            .astype(bf16))

    in_maps = []
    for c in range(n_cores):
        cols = slice(c * DL, (c + 1) * DL)
        woT = np.asarray(Wo, np.float32)[cols, :].T  # (D, DL)
        in_maps.append({
            "xT": xT,
            "x_colsT": np.ascontiguousarray(x[:, cols].T),
            "wq_t": pretile((np.asarray(Wq, np.float32)[cols, :] * ln_w).T),
            "wk_t": pretile((np.asarray(Wk, np.float32)[cols, :] * ln_w).T),
            "wv_t": pretile((np.asarray(Wv, np.float32)[cols, :] * ln_w).T),
            "wo_t": pretile(woT[perm, :]),
            "cosT": cosT,
            "sinT": sinT,
            "rot_t": rot_t,
        })
    return in_maps


_NC_CACHE = {}


def kernel(x, cos, sin, attention_mask, ln_w, Wq, Wk, Wv, Wo,
           _trace=False, _trace_cores=None):
    from concourse.bass_utils import run_bass_kernel_spmd

    cfg = CFG_FULL
    key = tuple(sorted(cfg.items()))
    if key not in _NC_CACHE:
        _NC_CACHE[key] = build_nc(**cfg)
    nc = _NC_CACHE[key]
    n_cores = cfg["n_cores"]
    in_maps = prepare_inputs(x, cos, sin, ln_w, Wq, Wk, Wv, Wo,
                             n_cores, cfg["heads_per_core"])
    res = run_bass_kernel_spmd(nc, in_maps, core_ids=list(range(n_cores)),
                               trace=_trace, trace_cores=_trace_cores)
    out = np.concatenate(
        [res.results[c]["out_colsT"].T for c in range(n_cores)], axis=1)
    kernel.last_result = res
    return out


# revision 22
# speedup vs baseline: 1.0593x; 1.0536x over previous
"""Trainium2 Bass kernel for a dense-transformer attention block.

Reference semantics (T=2048, D=2048, 16 heads, d_h=128):
    h = RMSNorm(x) * ln_w
    q,k,v = h @ W{q,k,v}.T  -> (n_h, T, d_h);  RoPE(q, k)
    att = softmax(causal(q k^T / sqrt(d_h))) @ v
    out = x + att @ Wo.T          (attention_mask is all-ones per spec)

Distribution: head-parallel over 8 cores (2 heads/core).  Each core:
  phase 1  QKV projections for its heads (bf16 matmuls, contract over d_model);
           RMSNorm folded in: row scales r[t] enter via r-scaled RoPE tables
           (q,k) and per-row scaling (v); ln_w is folded into the weights on
           the host.  rotate_half runs on the PE as a constant permutation
           matmul.  x^2 row-sums use fp8 squares + DoubleRow ones-matmuls
           (256-deep contraction per pass); the ones tile is 128 wide so the
           row-sums land broadcast across all partitions and the rsqrt
           Newton chain needs no partition broadcast (DVE is lane-parallel,
           so the wide compute costs nothing).
  phase 2  per-head causal attention with scores computed TRANSPOSED
           (S^T[j,i]) so no transposes are needed anywhere; probabilities
           exp to fp8 e4m3 (exp biased by -ln 32 so the max score stays
           inside e4m3 range; the 32x cancels in the softmax normalization);
           softmax row-sums accumulate broadcast on the PE via wide fp8
           DoubleRow ones-matmuls; A@V keeps bf16 V against fp8
           probabilities.
  phase 3  per-head AllGather of att^T rows.  collective_compute BLOCKS the
           gpsimd queue until the wire completes, so gpsimd carries ONLY the
           AllGather triggers (+ startup loads); everything else rides
           sync/scalar queues or the PE.
  phase 4  output projection column-shard, weight-stationary:
           out^T[:, cols_c] rows = sum_k WoT-chunk.T @ attT-chunk  + residual
           Interleaved into the main loop two blocks behind the collective so
           the AllGather latency (13-24us each, serialized on one CC stream)
           hides under later blocks' compute; the last block splits into ag
           halves so its head-0 half runs while head-1's gather flies.
Host assembles out = concat(out_colsT.T, axis=1).
"""

import math

import numpy as np

EPS = 1e-5
NEG = -1.0e30

CFG_FULL = dict(T=2048, D=2048, n_cores=8, heads_per_core=2)


# --------------------------------------------------------------------------
# device program
# --------------------------------------------------------------------------
def build_nc(T, D, n_cores, heads_per_core):
    import concourse.mybir as mybir
    import concourse.tile as tile
    from concourse import bacc

    DH = 128                      # head dim (hard-wired into layout)
    P = 128                       # partitions
    NH = heads_per_core
    DL = NH * DH                  # local width (q/k/v columns per core)
    KC = D // P                   # k-chunks over d_model
    TB = T // 512                 # 512-wide t blocks
    NIB = T // 512                # 512-wide i blocks
    NTS = T // P                  # 128-wide t subtiles
    f32 = mybir.dt.float32
    bf16 = mybir.dt.bfloat16
    fp8 = mybir.dt.float8e4
    i32 = mybir.dt.int32

    nc = bacc.Bacc("TRN2", target_bir_lowering=False, debug=False,
                   num_devices=n_cores)

    # ---- I/O ----
    xT = nc.dram_tensor("xT", [D, T], bf16, kind="ExternalInput").ap()
    xct_in = nc.dram_tensor("x_colsT", [DL, T], f32, kind="ExternalInput").ap()
    # weight tensors arrive host-pretiled in SBUF layout [P, KC*DL]
    wq_t = nc.dram_tensor("wq_t", [P, KC * DL], bf16, kind="ExternalInput").ap()
    wk_t = nc.dram_tensor("wk_t", [P, KC * DL], bf16, kind="ExternalInput").ap()
    wv_t = nc.dram_tensor("wv_t", [P, KC * DL], bf16, kind="ExternalInput").ap()
    # wo_t additionally row-permuted on host to the AllGather chunk order
    wo_t = nc.dram_tensor("wo_t", [P, KC * DL], bf16, kind="ExternalInput").ap()
    cosT = nc.dram_tensor("cosT", [DH, T], bf16, kind="ExternalInput").ap()
    sinT = nc.dram_tensor("sinT", [DH, T], bf16, kind="ExternalInput").ap()
    rot_t = nc.dram_tensor("rot_t", [DH, DH], bf16, kind="ExternalInput").ap()
    out_cT = nc.dram_tensor("out_colsT", [DL, T], f32,
                            kind="ExternalOutput").ap()

    Act = mybir.ActivationFunctionType
    Alu = mybir.AluOpType
    DR = mybir.MatmulPerfMode.DoubleRow
    inv_sqrt_dh = 1.0 / math.sqrt(DH)
    EXP_BIAS = -math.log(32.0)    # keeps exp() inside fp8 e4m3 range
    MAGIC = 0x5F3759DF

    with tile.TileContext(nc) as tc, \
            tc.tile_pool(name="persist", bufs=1) as persist:
        # ---------------- long-lived tensors ----------------
        Q_sb = persist.tile([P, NH, T], bf16, tag="Q_sb")
        K_sb = persist.tile([P, NH, T], bf16, tag="K_sb")
        V_sb = persist.tile([P, NTS, DL], bf16, tag="V_sb")
        rcol_sb = persist.tile([P, NTS], f32, tag="rcol_sb")
        # wide ones: row-sum matmuls produce their result broadcast across
        # all 128 partitions for free (streaming is rhs-bound)
        ones8 = persist.tile([P, 2, P], fp8, tag="ones8")
        masks_sb = persist.tile([P, 4, 512], f32, tag="masks_sb")
        rot_sb = persist.tile([P, DH], bf16, tag="rot_sb")
        ebias_sb = persist.tile([P, 1], f32, tag="ebias_sb")

        nc.gpsimd.dma_start(rot_sb[:], rot_t)
        nc.vector.memset(ebias_sb[:], EXP_BIAS)
        nc.vector.memset(ones8[:], 1.0)
        warm_sb = persist.tile([P, 128], bf16, tag="warm_sb")
        nc.vector.memset(warm_sb[:], 0.0)
        nc.gpsimd.memset(masks_sb[:], 0.0)
        for r in range(4):
            # keep (0) where i - j >= 0 with i = 512*B + f, j = 128*J + p,
            # offset r = J - 4*B  ->  f - p - 128 r >= 0
            nc.gpsimd.affine_select(
                out=masks_sb[:, r, :], in_=masks_sb[:, r, :],
                pattern=[[1, 512]], channel_multiplier=-1, base=-128 * r,
                compare_op=Alu.is_ge, fill=NEG)

        with tc.tile_pool(name="dram", bufs=1, space="DRAM") as dram_pool:
            ag_shared = "Shared" if n_cores > 4 else "Local"
            ag_in = [[dram_pool.tile([DH, 512], bf16, tag=f"agi{h}_{b}",
                                     name=f"ag_in{h}_{b}")
                      for b in range(NIB)] for h in range(NH)]
            ag_out = [[dram_pool.tile([n_cores * DH, 512], bf16,
                                      addr_space=ag_shared, tag=f"ago{h}_{b}",
                                      name=f"ag_out{h}_{b}")
                       for b in range(NIB)] for h in range(NH)]

            # PE warmup: ~5us of back-to-back dummy matmuls so the HAM
            # clock gate opens before the real work arrives
            with tc.tile_pool(name="warm_ps", bufs=1, space="PSUM") as wmps:
                wps = wmps.tile([P, 128], f32, tag="wm")
                for _ in range(40):
                    nc.tensor.matmul(wps[:], warm_sb[:], warm_sb[:],
                                     start=True, stop=True)

            # ==== phases 1+2+4 interleaved per t-block: QKV projections,
            # attention, the per-(head,block) all-gather, and the output
            # projection for the previous block — so the collective stream
            # starts early and its latency hides under compute.
            with (
                tc.tile_pool(name="wqkv", bufs=1) as wpool,
                tc.tile_pool(name="cs_raw", bufs=1) as cspool,
                tc.tile_pool(name="xk", bufs=1) as xpool,
                tc.tile_pool(name="sq", bufs=2) as sqpool,
                tc.tile_pool(name="tmp1", bufs=2) as tmppool,
                tc.tile_pool(name="rbc1", bufs=2) as rbcpool,
                tc.tile_pool(name="pt", bufs=3) as ptpool,
                tc.tile_pool(name="fin", bufs=2) as finpool,
                tc.tile_pool(name="ag_sb", bufs=8) as agpool,
                tc.tile_pool(name="xc", bufs=2) as xcpool,
                tc.tile_pool(name="osb", bufs=2) as opool,
                # PSUM: 4 + 3 + 1 banks (of 8)
                tc.tile_pool(name="big_ps", bufs=2, space="PSUM") as bigps,
                tc.tile_pool(name="sm_ps", bufs=3, space="PSUM") as smps,
                tc.tile_pool(name="row_ps", bufs=1, space="PSUM") as rowps,
            ):
                wq_sb = wpool.tile([P, KC, DL], bf16, tag="wq")
                wk_sb = wpool.tile([P, KC, DL], bf16, tag="wk")
                wv_sb = wpool.tile([P, KC, DL], bf16, tag="wv")
                wo_sb = wpool.tile([P, KC, DL], bf16, tag="wo")
                # interleave weight / x^T loads so the first q/k matmul can
                # start as soon as wq + xk[0] have landed
                xk = [xpool.tile([P, T], bf16, tag=f"xk{kc}", name=f"xk{kc}")
                      for kc in range(KC)]
                nc.sync.dma_start(wq_sb[:], wq_t.rearrange("p (kc j) -> p kc j", j=DL))
                for kc in range(KC):
                    eng = nc.sync if kc % 2 == 0 else nc.gpsimd
                    eng.dma_start(xk[kc][:], xT[P * kc:P * (kc + 1), :])
                nc.sync.dma_start(wk_sb[:], wk_t.rearrange("p (kc j) -> p kc j", j=DL))
                nc.sync.dma_start(wv_sb[:], wv_t.rearrange("p (kc j) -> p kc j", j=DL))
                nc.gpsimd.dma_start(wo_sb[:], wo_t.rearrange("p (kc j) -> p kc j", j=DL))
                # cos/sin tables; r is folded in per block, in place
                cos_r = cspool.tile([P, T], bf16, tag="cos")
                sin_r = cspool.tile([P, T], bf16, tag="sin")
                nc.sync.dma_start(cos_r[:], cosT)
                nc.sync.dma_start(sin_r[:], sinT)

                # ---------- phase 4 for block B (output projection) -------
                def phase4(B, halves=(0, 1)):
                    sl = slice(512 * B, 512 * B + 512)
                    oms = {}
                    for half in halves:
                        ags = []
                        for kc in range(8 * half, 8 * half + 8):
                            h_idx, c_idx = divmod(kc, KC // NH)
                            agt = agpool.tile([P, 512], bf16, tag="ag",
                                              name=f"ag{kc}_{B}")
                            eng = (nc.sync, nc.scalar)[kc % 2]
                            eng.dma_start(
                                agt[:],
                                ag_out[h_idx][B][P * c_idx:P * (c_idx + 1), :])
                            ags.append(agt)
                        for js in range(DL // P):
                            if half == halves[0]:
                                oms[js] = smps.tile([P, 512], f32, tag="sm",
                                                    name=f"om{js}_{B}")
                            om = oms[js]
                            for i, kc in enumerate(range(8 * half,
                                                         8 * half + 8)):
                                nc.tensor.matmul(
                                    om[:], wo_sb[:, kc, P * js:P * (js + 1)],
                                    ags[i][:],
                                    start=(half == halves[0] and i == 0),
                                    stop=(half == halves[-1] and i == 7))
                        if half != halves[-1]:
                            continue
                        for js in range(DL // P):
                            om = oms[js]
                            xct = xcpool.tile([P, 512], f32, tag="xct")
                            nc.scalar.dma_start(xct[:],
                                                xct_in[P * js:P * (js + 1), sl])
                            osb = opool.tile([P, 512], f32, tag="osb")
                            nc.vector.tensor_tensor(osb[:], om[:], xct[:],
                                                    Alu.add)
                            nc.sync.dma_start(out_cT[P * js:P * (js + 1), sl],
                                                osb[:])

                # ---------- phase 2 for block B, one head ----------
                def phase2_head(B, h):
                    ib = slice(512 * B, 512 * B + 512)
                    hs = slice(DH * h, DH * (h + 1))
                    av = smps.tile([P, 512], f32, tag="sm")
                    ssum = rowps.tile([P, 512], f32, tag="row")
                    Jmax = 4 * B + 3
                    for Jp in range(0, Jmax + 1, 2):
                        st = bigps.tile([P, 2, 512], f32, tag="big")
                        pt = ptpool.tile([P, 2, 512], fp8, tag="pt")
                        for gi in range(2):
                            J = Jp + gi
                            nc.tensor.matmul(st[:, gi, :],
                                             K_sb[:, h, P * J:P * (J + 1)],
                                             Q_sb[:, h, ib],
                                             start=True, stop=True)
                            if J // 4 == B:
                                nc.vector.tensor_tensor(
                                    st[:, gi, :], st[:, gi, :],
                                    masks_sb[:, J % 4, :], Alu.add)
                        nc.scalar.activation(pt[:], st[:], Act.Exp,
                                             scale=inv_sqrt_dh,
                                             bias=ebias_sb[:])
                        for gi in range(2):
                            J = Jp + gi
                            nc.tensor.matmul(av[:], V_sb[:, J, hs],
                                             pt[:, gi, :], start=(J == 0),
                                             stop=(J == Jmax))
                        nc.tensor.matmul(ssum[:], ones8[:], pt[:, 0:2, :],
                                         start=(Jp == 0), stop=(Jp == Jmax - 1),
                                         perf_mode=DR)
                    rinv = finpool.tile([P, 512], f32, tag="rinv")
                    nc.vector.reciprocal_approx_fast(rinv[:], ssum[:])
                    att = finpool.tile([P, 512], bf16, tag="att")
                    nc.vector.tensor_tensor(att[:], av[:], rinv[:], Alu.mult)
                    nc.sync.dma_start(ag_in[h][B][:], att[:])
                    nc.gpsimd.collective_compute(
                        "AllGather", Alu.bypass,
                        replica_groups=[list(range(n_cores))],
                        ins=[ag_in[h][B][:].opt()],
                        outs=[ag_out[h][B][:].opt()])

                for B in range(TB):
                    tb = slice(512 * B, 512 * B + 512)
                    # ---------- phase 1 for block B ----------
                    srow = rowps.tile([P, 512], f32, tag="row")
                    qps = bigps.tile([P, 2, 512], f32, tag="big")
                    for kc2 in range(0, KC, 2):
                        sq = sqpool.tile([P, 2, 512], fp8, tag="sq")
                        for gi in range(2):
                            nc.scalar.activation(sq[:, gi, :],
                                                 xk[kc2 + gi][:, tb],
                                                 Act.Square)
                        nc.tensor.matmul(srow[:], ones8[:], sq[:],
                                         start=(kc2 == 0),
                                         stop=(kc2 == KC - 2), perf_mode=DR)
                        for kc in (kc2, kc2 + 1):
                            for h in range(NH):
                                hs = slice(DH * h, DH * (h + 1))
                                nc.tensor.matmul(qps[:, h, :],
                                                 wq_sb[:, kc, hs],
                                                 xk[kc][:, tb],
                                                 start=(kc == 0),
                                                 stop=(kc == KC - 1))
                    for h in range(NH):
                        nc.vector.tensor_copy(Q_sb[:, h, tb], qps[:, h, :])
                    # r = rsqrt(mean + eps): bit-trick seed + 2 Newton (DVE);
                    # computed on all 128 partitions (srow arrives broadcast)
                    # so the result needs no partition broadcast anywhere
                    mrow = tmppool.tile([P, 512], f32, tag="mrow")
                    nc.vector.tensor_scalar(mrow[:], srow[:], 1.0 / D, EPS,
                                            Alu.mult, Alu.add)
                    ri = tmppool.tile([P, 512], i32, tag="ri")
                    nc.vector.tensor_scalar(ri[:], mrow[:].bitcast(i32), 1, None,
                                            Alu.arith_shift_right)
                    nc.vector.tensor_scalar(ri[:], ri[:], -1, MAGIC,
                                            Alu.mult, Alu.add)
                    rrv = ri[:].bitcast(f32)
                    tn = tmppool.tile([P, 512], f32, tag="tn")
                    nc.vector.tensor_tensor(tn[:], rrv, rrv, Alu.mult)
                    nc.vector.tensor_tensor(tn[:], tn[:], mrow[:], Alu.mult)
                    nc.vector.tensor_scalar(tn[:], tn[:], -0.5, 1.5,
                                            Alu.mult, Alu.add)
                    nc.vector.tensor_tensor(rrv, rrv, tn[:], Alu.mult)
                    nc.vector.tensor_tensor(tn[:], rrv, rrv, Alu.mult)
                    nc.vector.tensor_tensor(tn[:], tn[:], mrow[:], Alu.mult)
                    nc.vector.tensor_scalar(tn[:], tn[:], -0.5, 1.5,
                                            Alu.mult, Alu.add)
                    rbc = rbcpool.tile([P, 512], f32, tag="rbc")
                    nc.vector.tensor_tensor(rbc[:], rrv, tn[:], Alu.mult)
                    for s in range(4):
                        i = 4 * B + s
                        nc.scalar.dma_start(
                            out=rcol_sb[:, i:i + 1],
                            in_=rbc[0:1, 128 * s:128 * (s + 1)])
                    nc.vector.tensor_tensor(cos_r[:, tb], cos_r[:, tb], rbc[:], Alu.mult)
                    nc.vector.tensor_tensor(sin_r[:, tb], sin_r[:, tb], rbc[:], Alu.mult)
                    # K pass (second big-psum buffer; overlaps the Q drain)
                    kps = bigps.tile([P, 2, 512], f32, tag="big")
                    for kc in range(KC):
                        for h in range(NH):
                            hs = slice(DH * h, DH * (h + 1))
                            nc.tensor.matmul(kps[:, h, :], wk_sb[:, kc, hs],
                                             xk[kc][:, tb], start=(kc == 0),
                                             stop=(kc == KC - 1))
                    for h in range(NH):
                        nc.vector.tensor_copy(K_sb[:, h, tb], kps[:, h, :])
                    # V pass, one 512-row tile (1 psum bank) at a time
                    for ts in range(4):
                        i = 4 * B + ts
                        vp = smps.tile([P, 512], f32, tag="sm")
                        for kc in range(KC):
                            nc.tensor.matmul(vp[:, :DL],
                                             xk[kc][:, 512 * B + P * ts:
                                                    512 * B + P * (ts + 1)],
                                             wv_sb[:, kc, :], start=(kc == 0),
                                             stop=(kc == KC - 1))
                        nc.vector.tensor_copy(V_sb[:, i, :], vp[:, :DL])
                        nc.vector.tensor_scalar_mul(V_sb[:, i, :], V_sb[:, i, :],
                                                    rcol_sb[:, i:i + 1])
                    # RoPE in place on SBUF (r enters via the scaled tables)
                    for buf in (Q_sb, K_sb):
                        for h in range(NH):
                            qs = tmppool.tile([P, 512], bf16, tag="qs")
                            nc.vector.tensor_tensor(qs[:], buf[:, h, tb],
                                                    sin_r[:, tb], Alu.mult)
                            rps = smps.tile([P, 512], f32, tag="sm")
                            nc.tensor.matmul(rps[:], rot_sb[:], qs[:],
                                             start=True, stop=True)
                            nc.vector.tensor_tensor(buf[:, h, tb], buf[:, h, tb],
                                                    cos_r[:, tb], Alu.mult)
                            nc.vector.tensor_tensor(buf[:, h, tb], buf[:, h, tb],
                                                    rps[:], Alu.add)
                    # ---------- phase 2 + interleaved phase 4 ----------
                    phase2_head(B, 0)
                    if B == 2:
                        phase4(0)
                    phase2_head(B, 1)
                    if B == 2:
                        phase4(1)
                    elif B == 3:
                        phase4(2)
                # tail: only the last block's projection remains; its head-0
                # half queues ahead of the head-1 ag loads so it runs while
                # the second head's AllGather is still in flight
                phase4(TB - 1)

    nc.compile()
    return nc


# --------------------------------------------------------------------------
# host-side prep / entry point
# --------------------------------------------------------------------------
def prepare_inputs(x, cos, sin, ln_w, Wq, Wk, Wv, Wo, n_cores, heads_per_core):
    import ml_dtypes
    bf16 = ml_dtypes.bfloat16
    DH = 128
    DL = heads_per_core * DH
    x = np.ascontiguousarray(np.asarray(x, dtype=np.float32))
    cos = np.asarray(cos, dtype=np.float32)
    sin = np.asarray(sin, dtype=np.float32)
    ln_w = np.ascontiguousarray(np.asarray(ln_w, dtype=np.float32))
    xT = np.ascontiguousarray(x.T.astype(bf16))
    cosT = np.ascontiguousarray(cos.T.astype(bf16))
    sinT = np.ascontiguousarray(sin.T.astype(bf16))
    R = np.zeros((DH, DH), dtype=np.float32)
    R[np.arange(64), np.arange(64) + 64] = -1.0
    R[np.arange(64) + 64, np.arange(64)] = 1.0
    rot_t = np.ascontiguousarray(R.T.astype(bf16))
    # AllGather chunk order: head-major, then source core; each chunk is the
    # 128 att columns (global j = DL*c' + DH*h + d) that core c' / head h sent.
    perm = np.concatenate([
        DL * cp + DH * h + np.arange(DH)
        for h in range(heads_per_core) for cp in range(n_cores)
    ])
    D = x.shape[1]
    KC = D // DH

    def pretile(wT):
        # (D, DL) -> SBUF layout [P, KC*DL]: element (p, kc, j) = wT[128 kc + p, j]
        return np.ascontiguousarray(
            wT.reshape(KC, DH, DL).transpose(1, 0, 2).reshape(DH, KC * DL)
            .astype(bf16))

    in_maps = []
    for c in range(n_cores):
        cols = slice(c * DL, (c + 1) * DL)
        woT = np.asarray(Wo, np.float32)[cols, :].T  # (D, DL)
        in_maps.append({
            "xT": xT,
            "x_colsT": np.ascontiguousarray(x[:, cols].T),
            "wq_t": pretile((np.asarray(Wq, np.float32)[cols, :] * ln_w).T),
            "wk_t": pretile((np.asarray(Wk, np.float32)[cols, :] * ln_w).T),
            "wv_t": pretile((np.asarray(Wv, np.float32)[cols, :] * ln_w).T),
            "wo_t": pretile(woT[perm, :]),
            "cosT": cosT,
            "sinT": sinT,
            "rot_t": rot_t,
        })
    return in_maps


_NC_CACHE = {}


def kernel(x, cos, sin, attention_mask, ln_w, Wq, Wk, Wv, Wo,
           _trace=False, _trace_cores=None):
    from concourse.bass_utils import run_bass_kernel_spmd

    cfg = CFG_FULL
    key = tuple(sorted(cfg.items()))
    if key not in _NC_CACHE:
        _NC_CACHE[key] = build_nc(**cfg)
    nc = _NC_CACHE[key]
    n_cores = cfg["n_cores"]
    in_maps = prepare_inputs(x, cos, sin, ln_w, Wq, Wk, Wv, Wo,
                             n_cores, cfg["heads_per_core"])
    res = run_bass_kernel_spmd(nc, in_maps, core_ids=list(range(n_cores)),
                               trace=_trace, trace_cores=_trace_cores)
    out = np.concatenate(
        [res.results[c]["out_colsT"].T for c in range(n_cores)], axis=1)
    kernel.last_result = res
    return out


# revision 23
# speedup vs baseline: 1.0660x; 1.0063x over previous
"""Trainium2 Bass kernel for a dense-transformer attention block.

Reference semantics (T=2048, D=2048, 16 heads, d_h=128):
    h = RMSNorm(x) * ln_w
    q,k,v = h @ W{q,k,v}.T  -> (n_h, T, d_h);  RoPE(q, k)
    att = softmax(causal(q k^T / sqrt(d_h))) @ v
    out = x + att @ Wo.T          (attention_mask is all-ones per spec)

Distribution: head-parallel over 8 cores (2 heads/core).  Each core:
  phase 1  QKV projections for its heads (bf16 matmuls, contract over d_model);
           RMSNorm folded in: row scales r[t] enter via r-scaled RoPE tables
           (q,k) and per-row scaling (v); ln_w is folded into the weights on
           the host.  rotate_half runs on the PE as a constant permutation
           matmul.  x^2 row-sums use fp8 squares + DoubleRow ones-matmuls
           (256-deep contraction per pass); the ones tile is 128 wide so the
           row-sums land broadcast across all partitions and the rsqrt
           Newton chain needs no partition broadcast (DVE is lane-parallel,
           so the wide compute costs nothing).
  phase 2  per-head causal attention with scores computed TRANSPOSED
           (S^T[j,i]) so no transposes are needed anywhere; probabilities
           exp to fp8 e4m3 (exp biased by -ln 32 so the max score stays
           inside e4m3 range; the 32x cancels in the softmax normalization);
           softmax row-sums accumulate broadcast on the PE via wide fp8
           DoubleRow ones-matmuls; A@V keeps bf16 V against fp8
           probabilities.
  phase 3  per-head AllGather of att^T rows.  collective_compute BLOCKS the
           gpsimd queue until the wire completes, so gpsimd carries ONLY the
           AllGather triggers (+ startup loads); everything else rides
           sync/scalar queues or the PE.
  phase 4  output projection column-shard, weight-stationary:
           out^T[:, cols_c] rows = sum_k WoT-chunk.T @ attT-chunk  + residual
           Interleaved into the main loop two blocks behind the collective so
           the AllGather latency (13-24us each, serialized on one CC stream)
           hides under later blocks' compute; the last block splits into ag
           halves so its head-0 half runs while head-1's gather flies.
Host assembles out = concat(out_colsT.T, axis=1).
"""

import math

import numpy as np

EPS = 1e-5
NEG = -1.0e30

CFG_FULL = dict(T=2048, D=2048, n_cores=8, heads_per_core=2)


# --------------------------------------------------------------------------
# device program
# --------------------------------------------------------------------------
def build_nc(T, D, n_cores, heads_per_core):
    import concourse.mybir as mybir
    import concourse.tile as tile
    from concourse import bacc

    DH = 128                      # head dim (hard-wired into layout)
    P = 128                       # partitions
    NH = heads_per_core
    DL = NH * DH                  # local width (q/k/v columns per core)
    KC = D // P                   # k-chunks over d_model
    TB = T // 512                 # 512-wide t blocks
    NIB = T // 512                # 512-wide i blocks
    NTS = T // P                  # 128-wide t subtiles
    f32 = mybir.dt.float32
    bf16 = mybir.dt.bfloat16
    fp8 = mybir.dt.float8e4
    i32 = mybir.dt.int32

    nc = bacc.Bacc("TRN2", target_bir_lowering=False, debug=False,
                   num_devices=n_cores)

    # ---- I/O ----
    xT = nc.dram_tensor("xT", [D, T], bf16, kind="ExternalInput").ap()
    xct_in = nc.dram_tensor("x_colsT", [DL, T], f32, kind="ExternalInput").ap()
    # weight tensors arrive host-pretiled in SBUF layout [P, KC*DL]
    wq_t = nc.dram_tensor("wq_t", [P, KC * DL], bf16, kind="ExternalInput").ap()
    wk_t = nc.dram_tensor("wk_t", [P, KC * DL], bf16, kind="ExternalInput").ap()
    wv_t = nc.dram_tensor("wv_t", [P, KC * DL], bf16, kind="ExternalInput").ap()
    # wo_t additionally row-permuted on host to the AllGather chunk order
    wo_t = nc.dram_tensor("wo_t", [P, KC * DL], bf16, kind="ExternalInput").ap()
    cosT = nc.dram_tensor("cosT", [DH, T], bf16, kind="ExternalInput").ap()
    sinT = nc.dram_tensor("sinT", [DH, T], bf16, kind="ExternalInput").ap()
    rot_t = nc.dram_tensor("rot_t", [DH, DH], bf16, kind="ExternalInput").ap()
    out_cT = nc.dram_tensor("out_colsT", [DL, T], f32,
                            kind="ExternalOutput").ap()

    Act = mybir.ActivationFunctionType
    Alu = mybir.AluOpType
    DR = mybir.MatmulPerfMode.DoubleRow
    inv_sqrt_dh = 1.0 / math.sqrt(DH)
    EXP_BIAS = -math.log(32.0)    # keeps exp() inside fp8 e4m3 range
    MAGIC = 0x5F3759DF

    with tile.TileContext(nc) as tc, \
            tc.tile_pool(name="persist", bufs=1) as persist:
        # ---------------- long-lived tensors ----------------
        Q_sb = persist.tile([P, NH, T], bf16, tag="Q_sb")
        K_sb = persist.tile([P, NH, T], bf16, tag="K_sb")
        V_sb = persist.tile([P, NTS, DL], bf16, tag="V_sb")
        rcol_sb = persist.tile([P, NTS], f32, tag="rcol_sb")
        # wide ones: row-sum matmuls produce their result broadcast across
        # all 128 partitions for free (streaming is rhs-bound)
        ones8 = persist.tile([P, 2, P], fp8, tag="ones8")
        masks_sb = persist.tile([P, 4, 512], f32, tag="masks_sb")
        rot_sb = persist.tile([P, DH], bf16, tag="rot_sb")
        ebias_sb = persist.tile([P, 1], f32, tag="ebias_sb")

        nc.gpsimd.dma_start(rot_sb[:], rot_t)
        nc.vector.memset(ebias_sb[:], EXP_BIAS)
        nc.vector.memset(ones8[:], 1.0)
        warm_sb = persist.tile([P, 128], bf16, tag="warm_sb")
        nc.vector.memset(warm_sb[:], 0.0)
        nc.gpsimd.memset(masks_sb[:], 0.0)
        for r in range(4):
            # keep (0) where i - j >= 0 with i = 512*B + f, j = 128*J + p,
            # offset r = J - 4*B  ->  f - p - 128 r >= 0
            nc.gpsimd.affine_select(
                out=masks_sb[:, r, :], in_=masks_sb[:, r, :],
                pattern=[[1, 512]], channel_multiplier=-1, base=-128 * r,
                compare_op=Alu.is_ge, fill=NEG)

        with tc.tile_pool(name="dram", bufs=1, space="DRAM") as dram_pool:
            ag_shared = "Shared" if n_cores > 4 else "Local"
            ag_in = [[dram_pool.tile([DH, 512], bf16, tag=f"agi{h}_{b}",
                                     name=f"ag_in{h}_{b}")
                      for b in range(NIB)] for h in range(NH)]
            ag_out = [[dram_pool.tile([n_cores * DH, 512], bf16,
                                      addr_space=ag_shared, tag=f"ago{h}_{b}",
                                      name=f"ag_out{h}_{b}")
                       for b in range(NIB)] for h in range(NH)]

            # PE warmup: ~5us of back-to-back dummy matmuls so the HAM
            # clock gate opens before the real work arrives
            with tc.tile_pool(name="warm_ps", bufs=1, space="PSUM") as wmps:
                wps = wmps.tile([P, 128], f32, tag="wm")
                for _ in range(40):
                    nc.tensor.matmul(wps[:], warm_sb[:], warm_sb[:],
                                     start=True, stop=True)

            # ==== phases 1+2+4 interleaved per t-block: QKV projections,
            # attention, the per-(head,block) all-gather, and the output
            # projection for the previous block — so the collective stream
            # starts early and its latency hides under compute.
            with (
                tc.tile_pool(name="wqkv", bufs=1) as wpool,
                tc.tile_pool(name="cs_raw", bufs=1) as cspool,
                tc.tile_pool(name="xk", bufs=1) as xpool,
                tc.tile_pool(name="sq", bufs=2) as sqpool,
                tc.tile_pool(name="tmp1", bufs=2) as tmppool,
                tc.tile_pool(name="rbc1", bufs=2) as rbcpool,
                tc.tile_pool(name="pt", bufs=3) as ptpool,
                tc.tile_pool(name="fin", bufs=2) as finpool,
                tc.tile_pool(name="ag_sb", bufs=16) as agpool,
                tc.tile_pool(name="xc", bufs=2) as xcpool,
                tc.tile_pool(name="osb", bufs=2) as opool,
                # PSUM: 4 + 3 + 1 banks (of 8)
                tc.tile_pool(name="big_ps", bufs=2, space="PSUM") as bigps,
                tc.tile_pool(name="sm_ps", bufs=3, space="PSUM") as smps,
                tc.tile_pool(name="row_ps", bufs=1, space="PSUM") as rowps,
            ):
                wq_sb = wpool.tile([P, KC, DL], bf16, tag="wq")
                wk_sb = wpool.tile([P, KC, DL], bf16, tag="wk")
                wv_sb = wpool.tile([P, KC, DL], bf16, tag="wv")
                wo_sb = wpool.tile([P, KC, DL], bf16, tag="wo")
                # interleave weight / x^T loads so the first q/k matmul can
                # start as soon as wq + xk[0] have landed
                xk = [xpool.tile([P, T], bf16, tag=f"xk{kc}", name=f"xk{kc}")
                      for kc in range(KC)]
                nc.sync.dma_start(wq_sb[:], wq_t.rearrange("p (kc j) -> p kc j", j=DL))
                for kc in range(KC):
                    eng = nc.sync if kc % 2 == 0 else nc.gpsimd
                    eng.dma_start(xk[kc][:], xT[P * kc:P * (kc + 1), :])
                nc.sync.dma_start(wk_sb[:], wk_t.rearrange("p (kc j) -> p kc j", j=DL))
                nc.sync.dma_start(wv_sb[:], wv_t.rearrange("p (kc j) -> p kc j", j=DL))
                nc.gpsimd.dma_start(wo_sb[:], wo_t.rearrange("p (kc j) -> p kc j", j=DL))
                # cos/sin tables; r is folded in per block, in place
                cos_r = cspool.tile([P, T], bf16, tag="cos")
                sin_r = cspool.tile([P, T], bf16, tag="sin")
                nc.sync.dma_start(cos_r[:], cosT)
                nc.sync.dma_start(sin_r[:], sinT)

                # ---------- phase 4 for block B (output projection) -------
                # ag loads are issued separately from the matmuls: on the
                # gpsimd queue for interleaved blocks (it reaches them right
                # after the gather's wire completes, so they never park on a
                # semaphore and stall the queue for later DMAs), and early on
                # sync/scalar for the tail blocks.
                def phase4_loads(B, halves=(0, 1), engs=None):
                    ags = {}
                    for half in halves:
                        for kc in range(8 * half, 8 * half + 8):
                            h_idx, c_idx = divmod(kc, KC // NH)
                            agt = agpool.tile([P, 512], bf16, tag="ag",
                                              name=f"ag{kc}_{B}")
                            eng = (engs or (nc.sync, nc.scalar))[kc % 2]
                            eng.dma_start(
                                agt[:],
                                ag_out[h_idx][B][P * c_idx:P * (c_idx + 1), :])
                            ags[kc] = agt
                    return ags

                def phase4(B, halves=(0, 1), ags=None, engs=None):
                    sl = slice(512 * B, 512 * B + 512)
                    oms = {}
                    for half in halves:
                        if ags is None or not all(
                                kc in ags for kc in range(8 * half,
                                                          8 * half + 8)):
                            loaded = phase4_loads(B, halves=(half,), engs=engs)
                            if ags is None:
                                ags = loaded
                            else:
                                ags.update(loaded)
                        for js in range(DL // P):
                            if half == halves[0]:
                                oms[js] = smps.tile([P, 512], f32, tag="sm",
                                                    name=f"om{js}_{B}")
                            om = oms[js]
                            for i, kc in enumerate(range(8 * half,
                                                         8 * half + 8)):
                                nc.tensor.matmul(
                                    om[:], wo_sb[:, kc, P * js:P * (js + 1)],
                                    ags[kc][:],
                                    start=(half == halves[0] and i == 0),
                                    stop=(half == halves[-1] and i == 7))
                        if half != halves[-1]:
                            continue
                        for js in range(DL // P):
                            om = oms[js]
                            xct = xcpool.tile([P, 512], f32, tag="xct")
                            nc.scalar.dma_start(xct[:],
                                                xct_in[P * js:P * (js + 1), sl])
                            osb = opool.tile([P, 512], f32, tag="osb")
                            nc.vector.tensor_tensor(osb[:], om[:], xct[:],
                                                    Alu.add)
                            nc.sync.dma_start(out_cT[P * js:P * (js + 1), sl],
                                                osb[:])

                # ---------- phase 2 for block B, one head ----------
                def phase2_head(B, h):
                    ib = slice(512 * B, 512 * B + 512)
                    hs = slice(DH * h, DH * (h + 1))
                    av = smps.tile([P, 512], f32, tag="sm")
                    ssum = rowps.tile([P, 512], f32, tag="row")
                    Jmax = 4 * B + 3
                    for Jp in range(0, Jmax + 1, 2):
                        st = bigps.tile([P, 2, 512], f32, tag="big")
                        pt = ptpool.tile([P, 2, 512], fp8, tag="pt")
                        for gi in range(2):
                            J = Jp + gi
                            nc.tensor.matmul(st[:, gi, :],
                                             K_sb[:, h, P * J:P * (J + 1)],
                                             Q_sb[:, h, ib],
                                             start=True, stop=True)
                            if J // 4 == B:
                                nc.vector.tensor_tensor(
                                    st[:, gi, :], st[:, gi, :],
                                    masks_sb[:, J % 4, :], Alu.add)
                        nc.scalar.activation(pt[:], st[:], Act.Exp,
                                             scale=inv_sqrt_dh,
                                             bias=ebias_sb[:])
                        for gi in range(2):
                            J = Jp + gi
                            nc.tensor.matmul(av[:], V_sb[:, J, hs],
                                             pt[:, gi, :], start=(J == 0),
                                             stop=(J == Jmax))
                        nc.tensor.matmul(ssum[:], ones8[:], pt[:, 0:2, :],
                                         start=(Jp == 0), stop=(Jp == Jmax - 1),
                                         perf_mode=DR)
                    rinv = finpool.tile([P, 512], f32, tag="rinv")
                    nc.vector.reciprocal_approx_fast(rinv[:], ssum[:])
                    att = finpool.tile([P, 512], bf16, tag="att")
                    nc.vector.tensor_tensor(att[:], av[:], rinv[:], Alu.mult)
                    nc.sync.dma_start(ag_in[h][B][:], att[:])
                    nc.gpsimd.collective_compute(
                        "AllGather", Alu.bypass,
                        replica_groups=[list(range(n_cores))],
                        ins=[ag_in[h][B][:].opt()],
                        outs=[ag_out[h][B][:].opt()])

                for B in range(TB):
                    tb = slice(512 * B, 512 * B + 512)
                    # ---------- phase 1 for block B ----------
                    srow = rowps.tile([P, 512], f32, tag="row")
                    qps = bigps.tile([P, 2, 512], f32, tag="big")
                    for kc2 in range(0, KC, 2):
                        sq = sqpool.tile([P, 2, 512], fp8, tag="sq")
                        for gi in range(2):
                            nc.scalar.activation(sq[:, gi, :],
                                                 xk[kc2 + gi][:, tb],
                                                 Act.Square)
                        nc.tensor.matmul(srow[:], ones8[:], sq[:],
                                         start=(kc2 == 0),
                                         stop=(kc2 == KC - 2), perf_mode=DR)
                        for kc in (kc2, kc2 + 1):
                            for h in range(NH):
                                hs = slice(DH * h, DH * (h + 1))
                                nc.tensor.matmul(qps[:, h, :],
                                                 wq_sb[:, kc, hs],
                                                 xk[kc][:, tb],
                                                 start=(kc == 0),
                                                 stop=(kc == KC - 1))
                    for h in range(NH):
                        nc.vector.tensor_copy(Q_sb[:, h, tb], qps[:, h, :])
                    # r = rsqrt(mean + eps): bit-trick seed + 2 Newton (DVE);
                    # computed on all 128 partitions (srow arrives broadcast)
                    # so the result needs no partition broadcast anywhere
                    mrow = tmppool.tile([P, 512], f32, tag="mrow")
                    nc.vector.tensor_scalar(mrow[:], srow[:], 1.0 / D, EPS,
                                            Alu.mult, Alu.add)
                    ri = tmppool.tile([P, 512], i32, tag="ri")
                    nc.vector.tensor_scalar(ri[:], mrow[:].bitcast(i32), 1, None,
                                            Alu.arith_shift_right)
                    nc.vector.tensor_scalar(ri[:], ri[:], -1, MAGIC,
                                            Alu.mult, Alu.add)
                    rrv = ri[:].bitcast(f32)
                    tn = tmppool.tile([P, 512], f32, tag="tn")
                    nc.vector.tensor_tensor(tn[:], rrv, rrv, Alu.mult)
                    nc.vector.tensor_tensor(tn[:], tn[:], mrow[:], Alu.mult)
                    nc.vector.tensor_scalar(tn[:], tn[:], -0.5, 1.5,
                                            Alu.mult, Alu.add)
                    nc.vector.tensor_tensor(rrv, rrv, tn[:], Alu.mult)
                    nc.vector.tensor_tensor(tn[:], rrv, rrv, Alu.mult)
                    nc.vector.tensor_tensor(tn[:], tn[:], mrow[:], Alu.mult)
                    nc.vector.tensor_scalar(tn[:], tn[:], -0.5, 1.5,
                                            Alu.mult, Alu.add)
                    rbc = rbcpool.tile([P, 512], f32, tag="rbc")
                    nc.vector.tensor_tensor(rbc[:], rrv, tn[:], Alu.mult)
                    for s in range(4):
                        i = 4 * B + s
                        nc.scalar.dma_start(
                            out=rcol_sb[:, i:i + 1],
                            in_=rbc[0:1, 128 * s:128 * (s + 1)])
                    nc.vector.tensor_tensor(cos_r[:, tb], cos_r[:, tb], rbc[:], Alu.mult)
                    nc.vector.tensor_tensor(sin_r[:, tb], sin_r[:, tb], rbc[:], Alu.mult)
                    # K pass (second big-psum buffer; overlaps the Q drain)
                    kps = bigps.tile([P, 2, 512], f32, tag="big")
                    for kc in range(KC):
                        for h in range(NH):
                            hs = slice(DH * h, DH * (h + 1))
                            nc.tensor.matmul(kps[:, h, :], wk_sb[:, kc, hs],
                                             xk[kc][:, tb], start=(kc == 0),
                                             stop=(kc == KC - 1))
                    for h in range(NH):
                        nc.vector.tensor_copy(K_sb[:, h, tb], kps[:, h, :])
                    # V pass, one 512-row tile (1 psum bank) at a time
                    for ts in range(4):
                        i = 4 * B + ts
                        vp = smps.tile([P, 512], f32, tag="sm")
                        for kc in range(KC):
                            nc.tensor.matmul(vp[:, :DL],
                                             xk[kc][:, 512 * B + P * ts:
                                                    512 * B + P * (ts + 1)],
                                             wv_sb[:, kc, :], start=(kc == 0),
                                             stop=(kc == KC - 1))
                        nc.vector.tensor_copy(V_sb[:, i, :], vp[:, :DL])
                        nc.vector.tensor_scalar_mul(V_sb[:, i, :], V_sb[:, i, :],
                                                    rcol_sb[:, i:i + 1])
                    # RoPE in place on SBUF (r enters via the scaled tables)
                    for buf in (Q_sb, K_sb):
                        for h in range(NH):
                            qs = tmppool.tile([P, 512], bf16, tag="qs")
                            nc.vector.tensor_tensor(qs[:], buf[:, h, tb],
                                                    sin_r[:, tb], Alu.mult)
                            rps = smps.tile([P, 512], f32, tag="sm")
                            nc.tensor.matmul(rps[:], rot_sb[:], qs[:],
                                             start=True, stop=True)
                            nc.vector.tensor_tensor(buf[:, h, tb], buf[:, h, tb],
                                                    cos_r[:, tb], Alu.mult)
                            nc.vector.tensor_tensor(buf[:, h, tb], buf[:, h, tb],
                                                    rps[:], Alu.add)
                    # ---------- phase 2 + interleaved phase 4 ----------
                    if B == 3:
                        ags2 = phase4_loads(2)
                    phase2_head(B, 0)
                    if B == 2:
                        phase4(0, engs=(nc.gpsimd, nc.gpsimd))
                    phase2_head(B, 1)
                    if B == 2:
                        phase4(1, engs=(nc.gpsimd, nc.gpsimd))
                    elif B == 3:
                        phase4(2, ags=ags2)
                # tail: only the last block's projection remains; its head-0
                # half queues ahead of the head-1 ag loads so it runs while
                # the second head's AllGather is still in flight
                phase4(TB - 1)

    nc.compile()
    return nc


# --------------------------------------------------------------------------
# host-side prep / entry point
# --------------------------------------------------------------------------
def prepare_inputs(x, cos, sin, ln_w, Wq, Wk, Wv, Wo, n_cores, heads_per_core):
    import ml_dtypes
    bf16 = ml_dtypes.bfloat16
    DH = 128
    DL = heads_per_core * DH
    x = np.ascontiguousarray(np.asarray(x, dtype=np.float32))
    cos = np.asarray(cos, dtype=np.float32)
    sin = np.asarray(sin, dtype=np.float32)
    ln_w = np.ascontiguousarray(np.asarray(ln_w, dtype=np.float32))
    xT = np.ascontiguousarray(x.T.astype(bf16))
    cosT = np.ascontiguousarray(cos.T.astype(bf16))
    sinT = np.ascontiguousarray(sin.T.astype(bf16))
    R = np.zeros((DH, DH), dtype=np.float32)
    R[np.arange(64), np.arange(64) + 64] = -1.0
    R[np.arange(64) + 64, np.arange(64)] = 1.0
    rot_t = np.ascontiguousarray(R.T.astype(bf16))
    # AllGather chunk order: head-major, then source core; each chunk is the
    # 128 att columns (global j = DL*c' + DH*h + d) that core c' / head h sent.
    perm = np.concatenate([
        DL * cp + DH * h + np.arange(DH)
        for h in range(heads_per_core) for cp in range(n_cores)
    ])
    D = x.shape[1]
    KC = D // DH

    def pretile(wT):
        # (D, DL) -> SBUF layout [P, KC*DL]: element (p, kc, j) = wT[128 kc + p, j]
        return np.ascontiguousarray(
            wT.reshape(KC, DH, DL).transpose(1, 0, 2).reshape(DH, KC * DL)
            .astype(bf16))

    in_maps = []
    for c in range(n_cores):
        cols = slice(c * DL, (c + 1) * DL)
        woT = np.asarray(Wo, np.float32)[cols, :].T  # (D, DL)
        in_maps.append({
            "xT": xT,
            "x_colsT": np.ascontiguousarray(x[:, cols].T),
            "wq_t": pretile((np.asarray(Wq, np.float32)[cols, :] * ln_w).T),
            "wk_t": pretile((np.asarray(Wk, np.float32)[cols, :] * ln_w).T),
            "wv_t": pretile((np.asarray(Wv, np.float32)[cols, :] * ln_w).T),
            "wo_t": pretile(woT[perm, :]),
            "cosT": cosT,
            "sinT": sinT,
            "rot_t": rot_t,
        })
    return in_maps


_NC_CACHE = {}


def kernel(x, cos, sin, attention_mask, ln_w, Wq, Wk, Wv, Wo,
           _trace=False, _trace_cores=None):
    from concourse.bass_utils import run_bass_kernel_spmd

    cfg = CFG_FULL
    key = tuple(sorted(cfg.items()))
    if key not in _NC_CACHE:
        _NC_CACHE[key] = build_nc(**cfg)
    nc = _NC_CACHE[key]
    n_cores = cfg["n_cores"]
    in_maps = prepare_inputs(x, cos, sin, ln_w, Wq, Wk, Wv, Wo,
                             n_cores, cfg["heads_per_core"])
    res = run_bass_kernel_spmd(nc, in_maps, core_ids=list(range(n_cores)),
                               trace=_trace, trace_cores=_trace_cores)
    out = np.concatenate(
        [res.results[c]["out_colsT"].T for c in range(n_cores)], axis=1)
    kernel.last_result = res
    return out


# revision 24
# speedup vs baseline: 1.1293x; 1.0593x over previous
"""Trainium2 Bass kernel for a dense-transformer attention block.

Reference semantics (T=2048, D=2048, 16 heads, d_h=128):
    h = RMSNorm(x) * ln_w
    q,k,v = h @ W{q,k,v}.T  -> (n_h, T, d_h);  RoPE(q, k)
    att = softmax(causal(q k^T / sqrt(d_h))) @ v
    out = x + att @ Wo.T          (attention_mask is all-ones per spec)

Distribution: head-parallel over 8 cores (2 heads/core).  Each core:
  phase 1  QKV projections for its heads (bf16 matmuls, contract over d_model);
           RMSNorm folded in: row scales r[t] enter via r-scaled RoPE tables
           (q,k) and per-row scaling (v); ln_w is folded into the weights on
           the host.  rotate_half runs on the PE as a constant permutation
           matmul.  x^2 row-sums use fp8 squares + DoubleRow ones-matmuls
           (256-deep contraction per pass); the ones tile is 128 wide so the
           row-sums land broadcast across all partitions and the rsqrt
           Newton chain needs no partition broadcast (DVE is lane-parallel,
           so the wide compute costs nothing).
  phase 2  per-head causal attention with scores computed TRANSPOSED
           (S^T[j,i]) so no transposes are needed anywhere; probabilities
           exp to fp8 e4m3 (exp biased by -ln 32 so the max score stays
           inside e4m3 range; the 32x cancels in the softmax normalization);
           softmax row-sums accumulate broadcast on the PE via wide fp8
           DoubleRow ones-matmuls; A@V keeps bf16 V against fp8
           probabilities.
  phase 3  per-head AllGather of att^T rows.  collective_compute BLOCKS the
           gpsimd queue until the wire completes, so gpsimd carries ONLY the
           AllGather triggers (+ startup loads); everything else rides
           sync/scalar queues or the PE.
  phase 4  output projection column-shard, weight-stationary:
           out^T[:, cols_c] rows = sum_k WoT-chunk.T @ attT-chunk  + residual
           Interleaved into the main loop two blocks behind the collective so
           the AllGather latency (13-24us each, serialized on one CC stream)
           hides under later blocks' compute; the last block splits into ag
           halves so its head-0 half runs while head-1's gather flies.
Host assembles out = concat(out_colsT.T, axis=1).
"""

import math

import numpy as np

EPS = 1e-5
NEG = -1.0e30

CFG_FULL = dict(T=2048, D=2048, n_cores=8, heads_per_core=2)


# --------------------------------------------------------------------------
# device program
# --------------------------------------------------------------------------
def build_nc(T, D, n_cores, heads_per_core):
    import concourse.mybir as mybir
    import concourse.tile as tile
    from concourse import bacc

    DH = 128                      # head dim (hard-wired into layout)
    P = 128                       # partitions
    NH = heads_per_core
    DL = NH * DH                  # local width (q/k/v columns per core)
    KC = D // P                   # k-chunks over d_model
    TB = T // 512                 # 512-wide t blocks
    NIB = T // 512                # 512-wide i blocks
    NTS = T // P                  # 128-wide t subtiles
    f32 = mybir.dt.float32
    bf16 = mybir.dt.bfloat16
    fp8 = mybir.dt.float8e4
    i32 = mybir.dt.int32

    nc = bacc.Bacc("TRN2", target_bir_lowering=False, debug=False,
                   num_devices=n_cores)

    # ---- I/O ----
    xT = nc.dram_tensor("xT", [D, T], bf16, kind="ExternalInput").ap()
    xct_in = nc.dram_tensor("x_colsT", [DL, T], f32, kind="ExternalInput").ap()
    # weight tensors arrive host-pretiled in SBUF layout [P, KC*DL]
    wq_t = nc.dram_tensor("wq_t", [P, KC * DL], bf16, kind="ExternalInput").ap()
    wk_t = nc.dram_tensor("wk_t", [P, KC * DL], bf16, kind="ExternalInput").ap()
    wv_t = nc.dram_tensor("wv_t", [P, KC * DL], bf16, kind="ExternalInput").ap()
    # wo_t additionally row-permuted on host to the AllGather chunk order
    wo_t = nc.dram_tensor("wo_t", [P, KC * DL], bf16, kind="ExternalInput").ap()
    cosT = nc.dram_tensor("cosT", [DH, T], bf16, kind="ExternalInput").ap()
    sinT = nc.dram_tensor("sinT", [DH, T], bf16, kind="ExternalInput").ap()
    rot_t = nc.dram_tensor("rot_t", [DH, DH], bf16, kind="ExternalInput").ap()
    out_cT = nc.dram_tensor("out_colsT", [DL, T], f32,
                            kind="ExternalOutput").ap()

    Act = mybir.ActivationFunctionType
    Alu = mybir.AluOpType
    DR = mybir.MatmulPerfMode.DoubleRow
    inv_sqrt_dh = 1.0 / math.sqrt(DH)
    EXP_BIAS = -math.log(32.0)    # keeps exp() inside fp8 e4m3 range
    MAGIC = 0x5F3759DF

    with tile.TileContext(nc) as tc, \
            tc.tile_pool(name="persist", bufs=1) as persist:
        # ---------------- long-lived tensors ----------------
        Q_sb = persist.tile([P, NH, T], bf16, tag="Q_sb")
        K_sb = persist.tile([P, NH, T], bf16, tag="K_sb")
        V_sb = persist.tile([P, NTS, DL], bf16, tag="V_sb")
        rcol_sb = persist.tile([P, NTS], f32, tag="rcol_sb")
        # wide ones: row-sum matmuls produce their result broadcast across
        # all 128 partitions for free (streaming is rhs-bound)
        ones8 = persist.tile([P, 2, P], fp8, tag="ones8")
        masks_sb = persist.tile([P, 4, 512], f32, tag="masks_sb")
        rot_sb = persist.tile([P, DH], bf16, tag="rot_sb")
        ebias_sb = persist.tile([P, 1], f32, tag="ebias_sb")

        nc.gpsimd.dma_start(rot_sb[:], rot_t)
        nc.vector.memset(ebias_sb[:], EXP_BIAS)
        nc.vector.memset(ones8[:], 1.0)
        warm_sb = persist.tile([P, 128], bf16, tag="warm_sb")
        nc.vector.memset(warm_sb[:], 0.0)
        nc.gpsimd.memset(masks_sb[:], 0.0)
        for r in range(4):
            # keep (0) where i - j >= 0 with i = 512*B + f, j = 128*J + p,
            # offset r = J - 4*B  ->  f - p - 128 r >= 0
            nc.gpsimd.affine_select(
                out=masks_sb[:, r, :], in_=masks_sb[:, r, :],
                pattern=[[1, 512]], channel_multiplier=-1, base=-128 * r,
                compare_op=Alu.is_ge, fill=NEG)

        with tc.tile_pool(name="dram", bufs=1, space="DRAM") as dram_pool:
            ag_shared = "Shared" if n_cores > 4 else "Local"
            ag_in = [[dram_pool.tile([DH, 512], bf16, tag=f"agi{h}_{b}",
                                     name=f"ag_in{h}_{b}")
                      for b in range(NIB)] for h in range(NH)]
            ag_out = [[dram_pool.tile([n_cores * DH, 512], bf16,
                                      addr_space=ag_shared, tag=f"ago{h}_{b}",
                                      name=f"ag_out{h}_{b}")
                       for b in range(NIB)] for h in range(NH)]

            # PE warmup: ~5us of back-to-back dummy matmuls so the HAM
            # clock gate opens before the real work arrives
            with tc.tile_pool(name="warm_ps", bufs=1, space="PSUM") as wmps:
                wps = wmps.tile([P, 128], f32, tag="wm")
                for _ in range(40):
                    nc.tensor.matmul(wps[:], warm_sb[:], warm_sb[:],
                                     start=True, stop=True)

            # ==== phases 1+2+4 interleaved per t-block: QKV projections,
            # attention, the per-(head,block) all-gather, and the output
            # projection for the previous block — so the collective stream
            # starts early and its latency hides under compute.
            with (
                tc.tile_pool(name="wqkv", bufs=1) as wpool,
                tc.tile_pool(name="cs_raw", bufs=1) as cspool,
                tc.tile_pool(name="xk", bufs=1) as xpool,
                tc.tile_pool(name="sq", bufs=2) as sqpool,
                tc.tile_pool(name="tmp1", bufs=2) as tmppool,
                tc.tile_pool(name="rbc1", bufs=2) as rbcpool,
                tc.tile_pool(name="pt", bufs=3) as ptpool,
                tc.tile_pool(name="fin", bufs=2) as finpool,
                tc.tile_pool(name="ag_sb", bufs=16) as agpool,
                tc.tile_pool(name="xc", bufs=2) as xcpool,
                tc.tile_pool(name="osb", bufs=2) as opool,
                # PSUM: 4 + 3 + 1 banks (of 8)
                tc.tile_pool(name="big_ps", bufs=2, space="PSUM") as bigps,
                tc.tile_pool(name="sm_ps", bufs=3, space="PSUM") as smps,
                tc.tile_pool(name="row_ps", bufs=1, space="PSUM") as rowps,
            ):
                wq_sb = wpool.tile([P, KC, DL], bf16, tag="wq")
                wk_sb = wpool.tile([P, KC, DL], bf16, tag="wk")
                wv_sb = wpool.tile([P, KC, DL], bf16, tag="wv")
                wo_sb = wpool.tile([P, KC, DL], bf16, tag="wo")
                # interleave weight / x^T loads so the first q/k matmul can
                # start as soon as wq + xk[0] have landed
                xk = [xpool.tile([P, T], bf16, tag=f"xk{kc}", name=f"xk{kc}")
                      for kc in range(KC)]
                nc.sync.dma_start(wq_sb[:], wq_t.rearrange("p (kc j) -> p kc j", j=DL))
                for kc in range(KC):
                    eng = nc.sync if kc % 2 == 0 else nc.gpsimd
                    eng.dma_start(xk[kc][:], xT[P * kc:P * (kc + 1), :])
                nc.sync.dma_start(wk_sb[:], wk_t.rearrange("p (kc j) -> p kc j", j=DL))
                nc.sync.dma_start(wv_sb[:], wv_t.rearrange("p (kc j) -> p kc j", j=DL))
                nc.gpsimd.dma_start(wo_sb[:], wo_t.rearrange("p (kc j) -> p kc j", j=DL))
                # cos/sin tables; r is folded in per block, in place
                cos_r = cspool.tile([P, T], bf16, tag="cos")
                sin_r = cspool.tile([P, T], bf16, tag="sin")
                nc.sync.dma_start(cos_r[:], cosT)
                nc.sync.dma_start(sin_r[:], sinT)

                # ---------- phase 4 for block B (output projection) -------
                # ag loads are issued separately from the matmuls: on the
                # gpsimd queue for interleaved blocks (it reaches them right
                # after the gather's wire completes, so they never park on a
                # semaphore and stall the queue for later DMAs), and early on
                # sync/scalar for the tail blocks.
                def phase4_loads(B, halves=(0, 1), engs=None):
                    ags = {}
                    for half in halves:
                        for kc in range(8 * half, 8 * half + 8):
                            h_idx, c_idx = divmod(kc, KC // NH)
                            agt = agpool.tile([P, 512], bf16, tag="ag",
                                              name=f"ag{kc}_{B}")
                            eng = (engs or (nc.sync, nc.scalar))[kc % 2]
                            eng.dma_start(
                                agt[:],
                                ag_out[h_idx][B][P * c_idx:P * (c_idx + 1), :])
                            ags[kc] = agt
                    return ags

                def phase4(B, halves=(0, 1), ags=None, engs=None):
                    sl = slice(512 * B, 512 * B + 512)
                    oms = {}
                    for half in halves:
                        if ags is None or not all(
                                kc in ags for kc in range(8 * half,
                                                          8 * half + 8)):
                            loaded = phase4_loads(B, halves=(half,), engs=engs)
                            if ags is None:
                                ags = loaded
                            else:
                                ags.update(loaded)
                        for js in range(DL // P):
                            if half == halves[0]:
                                oms[js] = smps.tile([P, 512], f32, tag="sm",
                                                    name=f"om{js}_{B}")
                            om = oms[js]
                            for i, kc in enumerate(range(8 * half,
                                                         8 * half + 8)):
                                nc.tensor.matmul(
                                    om[:], wo_sb[:, kc, P * js:P * (js + 1)],
                                    ags[kc][:],
                                    start=(half == halves[0] and i == 0),
                                    stop=(half == halves[-1] and i == 7))
                        if half != halves[-1]:
                            continue
                        for js in range(DL // P):
                            om = oms[js]
                            xct = xcpool.tile([P, 512], f32, tag="xct")
                            nc.scalar.dma_start(xct[:],
                                                xct_in[P * js:P * (js + 1), sl])
                            osb = opool.tile([P, 512], f32, tag="osb")
                            nc.vector.tensor_tensor(osb[:], om[:], xct[:],
                                                    Alu.add)
                            nc.sync.dma_start(out_cT[P * js:P * (js + 1), sl],
                                                osb[:])

                # ---------- phase 2 for block B, one head ----------
                def phase2_head(B, h):
                    ib = slice(512 * B, 512 * B + 512)
                    hs = slice(DH * h, DH * (h + 1))
                    av = smps.tile([P, 512], f32, tag="sm")
                    ssum = rowps.tile([P, 512], f32, tag="row")
                    Jmax = 4 * B + 3
                    for Jp in range(0, Jmax + 1, 2):
                        st = bigps.tile([P, 2, 512], f32, tag="big")
                        pt = ptpool.tile([P, 2, 512], fp8, tag="pt")
                        for gi in range(2):
                            J = Jp + gi
                            nc.tensor.matmul(st[:, gi, :],
                                             K_sb[:, h, P * J:P * (J + 1)],
                                             Q_sb[:, h, ib],
                                             start=True, stop=True)
                            if J // 4 == B:
                                nc.vector.tensor_tensor(
                                    st[:, gi, :], st[:, gi, :],
                                    masks_sb[:, J % 4, :], Alu.add)
                        nc.scalar.activation(pt[:], st[:], Act.Exp,
                                             scale=inv_sqrt_dh,
                                             bias=ebias_sb[:])
                        for gi in range(2):
                            J = Jp + gi
                            nc.tensor.matmul(av[:], V_sb[:, J, hs],
                                             pt[:, gi, :], start=(J == 0),
                                             stop=(J == Jmax))
                        nc.tensor.matmul(ssum[:], ones8[:], pt[:, 0:2, :],
                                         start=(Jp == 0), stop=(Jp == Jmax - 1),
                                         perf_mode=DR)
                    rinv = finpool.tile([P, 512], f32, tag="rinv")
                    nc.vector.reciprocal_approx_fast(rinv[:], ssum[:])
                    att = finpool.tile([P, 512], bf16, tag="att")
                    nc.vector.tensor_tensor(att[:], av[:], rinv[:], Alu.mult)
                    nc.sync.dma_start(ag_in[h][B][:], att[:])
                    nc.gpsimd.collective_compute(
                        "AllGather", Alu.bypass,
                        replica_groups=[list(range(n_cores))],
                        ins=[ag_in[h][B][:].opt()],
                        outs=[ag_out[h][B][:].opt()])

                for B in range(TB):
                    tb = slice(512 * B, 512 * B + 512)
                    # ---------- phase 1 for block B ----------
                    srow = rowps.tile([P, 512], f32, tag="row")
                    qps = bigps.tile([P, 2, 512], f32, tag="big")
                    for kc2 in range(0, KC, 2):
                        sq = sqpool.tile([P, 2, 512], fp8, tag="sq")
                        for gi in range(2):
                            nc.scalar.activation(sq[:, gi, :],
                                                 xk[kc2 + gi][:, tb],
                                                 Act.Square)
                        nc.tensor.matmul(srow[:], ones8[:], sq[:],
                                         start=(kc2 == 0),
                                         stop=(kc2 == KC - 2), perf_mode=DR)
                        for kc in (kc2, kc2 + 1):
                            for h in range(NH):
                                hs = slice(DH * h, DH * (h + 1))
                                nc.tensor.matmul(qps[:, h, :],
                                                 wq_sb[:, kc, hs],
                                                 xk[kc][:, tb],
                                                 start=(kc == 0),
                                                 stop=(kc == KC - 1))
                    for h in range(NH):
                        nc.vector.tensor_copy(Q_sb[:, h, tb], qps[:, h, :])
                    # r = rsqrt(mean + eps): bit-trick seed + 2 Newton (DVE);
                    # computed on all 128 partitions (srow arrives broadcast)
                    # so the result needs no partition broadcast anywhere
                    mrow = tmppool.tile([P, 512], f32, tag="mrow")
                    nc.vector.tensor_scalar(mrow[:], srow[:], 1.0 / D, EPS,
                                            Alu.mult, Alu.add)
                    ri = tmppool.tile([P, 512], i32, tag="ri")
                    nc.vector.tensor_scalar(ri[:], mrow[:].bitcast(i32), 1, None,
                                            Alu.arith_shift_right)
                    nc.vector.tensor_scalar(ri[:], ri[:], -1, MAGIC,
                                            Alu.mult, Alu.add)
                    rrv = ri[:].bitcast(f32)
                    tn = tmppool.tile([P, 512], f32, tag="tn")
                    nc.vector.tensor_tensor(tn[:], rrv, rrv, Alu.mult)
                    nc.vector.tensor_tensor(tn[:], tn[:], mrow[:], Alu.mult)
                    nc.vector.tensor_scalar(tn[:], tn[:], -0.5, 1.5,
                                            Alu.mult, Alu.add)
                    nc.vector.tensor_tensor(rrv, rrv, tn[:], Alu.mult)
                    nc.vector.tensor_tensor(tn[:], rrv, rrv, Alu.mult)
                    nc.vector.tensor_tensor(tn[:], tn[:], mrow[:], Alu.mult)
                    nc.vector.tensor_scalar(tn[:], tn[:], -0.5, 1.5,
                                            Alu.mult, Alu.add)
                    rbc = rbcpool.tile([P, 512], f32, tag="rbc")
                    nc.vector.tensor_tensor(rbc[:], rrv, tn[:], Alu.mult)
                    for s in range(4):
                        i = 4 * B + s
                        nc.scalar.dma_start(
                            out=rcol_sb[:, i:i + 1],
                            in_=rbc[0:1, 128 * s:128 * (s + 1)])
                    nc.vector.tensor_tensor(cos_r[:, tb], cos_r[:, tb], rbc[:], Alu.mult)
                    nc.vector.tensor_tensor(sin_r[:, tb], sin_r[:, tb], rbc[:], Alu.mult)
                    # K pass (second big-psum buffer; overlaps the Q drain)
                    kps = bigps.tile([P, 2, 512], f32, tag="big")
                    for kc in range(KC):
                        for h in range(NH):
                            hs = slice(DH * h, DH * (h + 1))
                            nc.tensor.matmul(kps[:, h, :], wk_sb[:, kc, hs],
                                             xk[kc][:, tb], start=(kc == 0),
                                             stop=(kc == KC - 1))
                    for h in range(NH):
                        nc.vector.tensor_copy(K_sb[:, h, tb], kps[:, h, :])
                    # V pass, one 512-row tile (1 psum bank) at a time
                    for ts in range(4):
                        i = 4 * B + ts
                        vp = smps.tile([P, 512], f32, tag="sm")
                        for kc in range(KC):
                            nc.tensor.matmul(vp[:, :DL],
                                             xk[kc][:, 512 * B + P * ts:
                                                    512 * B + P * (ts + 1)],
                                             wv_sb[:, kc, :], start=(kc == 0),
                                             stop=(kc == KC - 1))
                        nc.vector.tensor_copy(V_sb[:, i, :], vp[:, :DL])
                        nc.vector.tensor_scalar_mul(V_sb[:, i, :], V_sb[:, i, :],
                                                    rcol_sb[:, i:i + 1])
                    # RoPE in place on SBUF (r enters via the scaled tables)
                    for buf in (Q_sb, K_sb):
                        for h in range(NH):
                            qs = tmppool.tile([P, 512], bf16, tag="qs")
                            nc.vector.tensor_tensor(qs[:], buf[:, h, tb],
                                                    sin_r[:, tb], Alu.mult)
                            rps = smps.tile([P, 512], f32, tag="sm")
                            nc.tensor.matmul(rps[:], rot_sb[:], qs[:],
                                             start=True, stop=True)
                            nc.vector.tensor_tensor(buf[:, h, tb], buf[:, h, tb],
                                                    cos_r[:, tb], Alu.mult)
                            nc.vector.tensor_tensor(buf[:, h, tb], buf[:, h, tb],
                                                    rps[:], Alu.add)
                    # ---------- phase 2 + interleaved phase 4 ----------
                    # ag loads for the previous projection issue before this
                    # block's attention: the queue pointers reach them after
                    # their gathers completed, so they fire without parking
                    # (a parked DMA stalls its whole engine queue).
                    if B == 2:
                        ags0 = phase4_loads(0)
                    elif B == 3:
                        ags2 = phase4_loads(2)
                    phase2_head(B, 0)
                    if B == 2:
                        phase4(0, ags=ags0)
                    phase2_head(B, 1)
                    if B == 2:
                        phase4(1)
                    elif B == 3:
                        phase4(2, ags=ags2)
                # tail: only the last block's projection remains; its head-0
                # half queues ahead of the head-1 ag loads so it runs while
                # the second head's AllGather is still in flight
                phase4(TB - 1)

    nc.compile()
    return nc


# --------------------------------------------------------------------------
# host-side prep / entry point
# --------------------------------------------------------------------------
def prepare_inputs(x, cos, sin, ln_w, Wq, Wk, Wv, Wo, n_cores, heads_per_core):
    import ml_dtypes
    bf16 = ml_dtypes.bfloat16
    DH = 128
    DL = heads_per_core * DH
    x = np.ascontiguousarray(np.asarray(x, dtype=np.float32))
    cos = np.asarray(cos, dtype=np.float32)
    sin = np.asarray(sin, dtype=np.float32)
    ln_w = np.ascontiguousarray(np.asarray(ln_w, dtype=np.float32))
    xT = np.ascontiguousarray(x.T.astype(bf16))
    cosT = np.ascontiguousarray(cos.T.astype(bf16))
    sinT = np.ascontiguousarray(sin.T.astype(bf16))
    R = np.zeros((DH, DH), dtype=np.float32)
    R[np.arange(64), np.arange(64) + 64] = -1.0
    R[np.arange(64) + 64, np.arange(64)] = 1.0
    rot_t = np.ascontiguousarray(R.T.astype(bf16))
    # AllGather chunk order: head-major, then source core; each chunk is the
    # 128 att columns (global j = DL*c' + DH*h + d) that core c' / head h sent.
    perm = np.concatenate([
        DL * cp + DH * h + np.arange(DH)
        for h in range(heads_per_core) for cp in range(n_cores)
    ])
    D = x.shape[1]
    KC = D // DH

    def pretile(wT):
        # (D, DL) -> SBUF layout [P, KC*DL]: element (p, kc, j) = wT[128 kc + p, j]
        return np.ascontiguousarray(
            wT.reshape(KC, DH, DL).transpose(1, 0, 2).reshape(DH, KC * DL)
            .astype(bf16))

    in_maps = []
    for c in range(n_cores):
        cols = slice(c * DL, (c + 1) * DL)
        woT = np.asarray(Wo, np.float32)[cols, :].T  # (D, DL)
        in_maps.append({
            "xT": xT,
            "x_colsT": np.ascontiguousarray(x[:, cols].T),
            "wq_t": pretile((np.asarray(Wq, np.float32)[cols, :] * ln_w).T),
            "wk_t": pretile((np.asarray(Wk, np.float32)[cols, :] * ln_w).T),
            "wv_t": pretile((np.asarray(Wv, np.float32)[cols, :] * ln_w).T),
            "wo_t": pretile(woT[perm, :]),
            "cosT": cosT,
            "sinT": sinT,
            "rot_t": rot_t,
        })
    return in_maps


_NC_CACHE = {}


def kernel(x, cos, sin, attention_mask, ln_w, Wq, Wk, Wv, Wo,
           _trace=False, _trace_cores=None):
    from concourse.bass_utils import run_bass_kernel_spmd

    cfg = CFG_FULL
    key = tuple(sorted(cfg.items()))
    if key not in _NC_CACHE:
        _NC_CACHE[key] = build_nc(**cfg)
    nc = _NC_CACHE[key]
    n_cores = cfg["n_cores"]
    in_maps = prepare_inputs(x, cos, sin, ln_w, Wq, Wk, Wv, Wo,
                             n_cores, cfg["heads_per_core"])
    res = run_bass_kernel_spmd(nc, in_maps, core_ids=list(range(n_cores)),
                               trace=_trace, trace_cores=_trace_cores)
    out = np.concatenate(
        [res.results[c]["out_colsT"].T for c in range(n_cores)], axis=1)
    kernel.last_result = res
    return out
